# revision 5
# baseline (speedup 1.0000x reference)
"""ConvLSTM cell (complex-valued gates) on 8 TRN2 NeuronCores.

Strategy
--------
Data-parallel over batch: 16 images -> 2 per core. Per core, the three
live gates' complex 3x3 convs are computed as shifted matmuls
accumulated in PSUM:

    out[128, 512] += lhsT[128in, 128out].T @ z_shift[128in, 512]

The i and o gates (both sigmoid) are computed with the Gauss 3-mult
complex trick, packed pairwise so every pass keeps the full 128-wide
PE output:

    t1 = conv(zr+zi, Wr)        t2 = conv(zr, Wi-Wr)   t3 = conv(zi, Wr+Wi)
    y_r = t1 - t3               y_i = t1 + t2

with lhsT halves [t_i | t_o]. That is 3 passes per tap for both gates
vs 4 direct. The c gate stays direct (2 passes/tap, [re|im] packed).
Total 45 PE passes/tap-set vs 54 direct: ~154 us tensor-engine floor.

All matmul operands are fp16 (full PE speed). ScalarE applies
sigmoid/tanh from PSUM/SBUF with the per-channel bias fused. VectorE
does the Gauss combines (PSUM->SBUF) and the complex elementwise update
in fp16. x (*) c_prev is an input-only elementwise term precomputed on
the host and added on-chip. Outputs leave as fp16, upcast on host.

The spatial dim is processed in 10 macro-tiles per core (8..16 rows x
1024 cols max), 4 PSUM accumulation tiles (t1/t2/t3/c) per chunk.
z is kept resident in SBUF, zero-padded to 66x66 on the host so conv
taps are plain shifted access patterns. Weight DMA is split into
per-stream blocks in first-use order so the first matmul starts early.
"""
import sys
import numpy as np

sys.path.insert(0, "/opt/trn_rl_repo")

P = 128          # partitions / channels (64 real + 64 imag)
HALF = 64
B = 16           # full batch
N_CORES = 8
B_CORE = B // N_CORES   # batch per core
H = W = 64
HP = WP = 66     # padded spatial
MACRO = 16 * W   # max columns per macro tile
NM = 45          # packed weight passes: 9 taps x (3 io-gauss + 2 c-direct)

_CACHE = {}


def _apply_drain_patch(tile_mod):
    """The kernel-tail drain aggregates one wait per live proc-semaphore, but
    walrus rejects instructions with more than a few sync waits. Split the
    tail waits across a chain of single-wait drains."""
    if getattr(tile_mod.TileContext, "_drain_patched", False):
        return

    def _patched(self, tick_clock, wait_clock):
        ScopedClock = tile_mod.ScopedClock
        nc = self.nc
        drain_inst = nc.sync.drain()
        wait_clock.add_sem_waits(
            drain_inst.ins, ScopedClock({None: tick_clock.global_clock})
        )
        NW = 3
        si = drain_inst.ins.sync_info
        if si is not None and si.on_wait and len(si.on_wait) > NW:
            conds = list(si.on_wait)
            si.on_wait = conds[:NW]
            rest = conds[NW:]
            while rest:
                extra = nc.sync.drain()
                esi = extra.ins.sync_info
                if esi is None:
                    import bass_rust
                    extra.ins.sync_info = bass_rust.SyncInfo(
                        on_wait=rest[:NW], on_update=[])
                else:
                    esi.on_wait = rest[:NW]
                rest = rest[NW:]

        nc.all_engine_barrier()
        assert self.sems is not None
        popped = nc._tile_sem_poison_stack.pop()
        assert popped is self._sem_poison
        nc.clear_and_free_semaphores(list(self.sems.allocated().values()))
        nc.all_engine_barrier()

    tile_mod.TileContext._drain_and_barrier = _patched
    tile_mod.TileContext._drain_patched = True


def _split_excess_waits(nc, max_waits=1):
    """walrus's per-instruction sync-wait slots are tight (1 for some ISA
    structs). Hoist excess waits into same-engine no-ops inserted directly
    before the instruction — identical semantics, per-engine order kept."""
    import concourse.mybir as mybir
    n_new = 0
    for fn in nc.m.functions:
        for bb in fn.blocks:
            il = bb.instructions
            out = []
            for inst in il:
                si = inst.sync_info
                if si is not None and si.on_wait and len(si.on_wait) > max_waits:
                    conds = list(si.on_wait)
                    si.on_wait = conds[:max_waits]
                    rest = conds[max_waits:]
                    for j in range(0, len(rest), max_waits):
                        nop = mybir.InstNoOp(
                            name=f"{inst.name}_w{j}",
                            sync_info=mybir.SyncInfo(
                                on_wait=rest[j:j + max_waits], on_update=[]),
                            bass_nofuse=True,
                            engine=inst.engine,
                        )
                        out.append(nop)
                        n_new += 1
                out.append(inst)
            if n_new:
                il[:] = out
    return n_new


def _build_program():
    import concourse.bass as bass
    import concourse.tile as tile
    from concourse import mybir
    from contextlib import ExitStack

    _apply_drain_patch(tile)
    fp16 = mybir.dt.float16
    f32 = mybir.dt.float32
    Sigmoid = mybir.ActivationFunctionType.Sigmoid
    Tanh = mybir.ActivationFunctionType.Tanh
    Copy = mybir.ActivationFunctionType.Copy

    nc = bass.Bass("TRN2", target_bir_lowering=False, debug=False)
    zs_d = nc.dram_tensor("zs", [P, B_CORE, HP, WP], fp16, kind="ExternalInput").ap()
    zr_d = nc.dram_tensor("zr", [P, B_CORE, HP, WP], fp16, kind="ExternalInput").ap()
    zi_d = nc.dram_tensor("zi", [P, B_CORE, HP, WP], fp16, kind="ExternalInput").ap()
    w_d = nc.dram_tensor("wts", [P, NM, P], fp16, kind="ExternalInput").ap()
    b_d = nc.dram_tensor("bias", [P, 6], f32, kind="ExternalInput").ap()
    xc_d = nc.dram_tensor("xc", [P, B_CORE, H, W], fp16, kind="ExternalInput").ap()
    h_d = nc.dram_tensor("h_out", [P, B_CORE, H, W], fp16, kind="ExternalOutput").ap()
    c_d = nc.dram_tensor("c_out", [P, B_CORE, H, W], fp16, kind="ExternalOutput").ap()

    # padded-row chunks (overlapping): A=[0:18) B=[16:34) C=[32:66)
    Z_CHUNKS = {'A': (0, 18), 'B': (16, 18), 'C': (32, 34)}
    # matmul streams: (z component, weight-block offset). io-gauss t1/t2/t3
    # then c-direct zr/zi; per block 9 taps.
    STREAMS = [('s', 0), ('r', 9), ('i', 18), ('r', 27), ('i', 36)]

    with tile.TileContext(nc) as tc, ExitStack() as ctx:
        const = ctx.enter_context(tc.tile_pool(name="const", bufs=1))
        w_s = const.tile([P, NM, P], fp16, name="wts")
        z_ch = {}

        def load_w(blk, after=None):
            dm = nc.sync.dma_start(w_s[:, blk * 9:(blk + 1) * 9, :],
                                   w_d[:, blk * 9:(blk + 1) * 9, :])
            if after is not None:
                tile.add_dep_helper(dm.ins, after, reason="defer w block")

        def load_z(b, ch, comps='sri', after=None):
            row0, nr = Z_CHUNKS[ch]
            for comp in comps:
                zt_d = {'s': zs_d, 'r': zr_d, 'i': zi_d}[comp]
                t = const.tile([P, nr, WP], fp16, name=f"z{comp}_{b}_{ch}")
                dm = nc.sync.dma_start(t[:], zt_d[:, b, row0:row0 + nr, :])
                if after is not None:
                    # hold the transfer back until the anchor matmul retires so
                    # it can't steal HBM bandwidth from earlier-needed loads
                    tile.add_dep_helper(dm.ins, after,
                                        reason="defer non-critical z load")
                z_ch[(comp, b, ch)] = t

        # upfront loads in first-consumption order so the first macro-tile's
        # matmuls start after ~0.6MB of DMA instead of ~4MB
        load_w(0)
        load_z(0, 'A', comps='s')
        load_z(0, 'A', comps='r')
        load_w(1)
        load_z(0, 'A', comps='i')
        load_w(2)
        load_w(3)
        load_w(4)
        bias_s = const.tile([P, 6], f32)
        nc.sync.dma_start(bias_s[:], b_d[:])

        ps_1 = ctx.enter_context(tc.tile_pool(name="ps_1", bufs=1, space="PSUM"))
        ps_2 = ctx.enter_context(tc.tile_pool(name="ps_2", bufs=1, space="PSUM"))
        ps_3 = ctx.enter_context(tc.tile_pool(name="ps_3", bufs=1, space="PSUM"))
        ps_c = ctx.enter_context(tc.tile_pool(name="ps_c", bufs=1, space="PSUM"))
        work = ctx.enter_context(tc.tile_pool(name="work", bufs=2))

        def macro_tile(b, r0, nrows):
            cols = nrows * W
            if r0 + nrows + 1 < 18:
                ch = 'A'
            elif r0 >= 16 and r0 + nrows + 1 < 34:
                ch = 'B'
            else:
                ch = 'C'
            roff = Z_CHUNKS[ch][0]   # chunk's first padded row

            last_mm = [None]

            def conv_block(pt, si, start, stop):
                comp, mbase = STREAMS[si]
                z_s = z_ch[(comp, b, ch)]
                for t in range(9):
                    kh, kw = t // 3, t % 3
                    for half in range(nrows // 8):
                        r0h = r0 + half * 8 - roff
                        mm = nc.tensor.matmul(
                            pt[:, half * 512:(half + 1) * 512],
                            w_s[:, mbase + t, :],
                            z_s[:, r0h + kh:r0h + kh + 8, kw:kw + 64],
                            start=(start and t == 0), stop=(stop and t == 8),
                        )
                        last_mm[0] = mm.ins
                return pt

            # io gauss unit: t1/t2/t3 accumulators, stream-major so each
            # weight block + z component is consumed in DMA arrival order
            T1 = ps_1.tile([P, cols], f32, tag="pt1")
            T2 = ps_2.tile([P, cols], f32, tag="pt2")
            T3 = ps_3.tile([P, cols], f32, tag="pt3")
            conv_block(T1, 0, True, True)
            conv_block(T2, 1, True, True)
            conv_block(T3, 2, True, True)

            # gauss combines: DVE can read only one PSUM operand, so stage t1
            # through SBUF via ScalarE first; halves stay base-aligned
            T1s = work.tile([P, cols], fp16, tag="T1s")
            nc.scalar.activation(T1s[:], T1[:], Copy)
            tA = work.tile([P, cols], fp16, tag="tA")   # [i_r pre ; o_r pre]
            tB = work.tile([P, cols], fp16, tag="tB")   # [i_i pre ; o_i pre]
            nc.vector.tensor_sub(tA[0:HALF, :], T1s[0:HALF, :], T3[0:HALF, :])
            nc.vector.tensor_add(tB[0:HALF, :], T1s[0:HALF, :], T2[0:HALF, :])
            nc.vector.tensor_sub(tA[HALF:P, :], T1s[HALF:P, :], T3[HALF:P, :])
            nc.vector.tensor_add(tB[HALF:P, :], T1s[HALF:P, :], T2[HALF:P, :])

            # sigmoids with fused bias; ScalarE may cross partition bases
            I = work.tile([P, cols], fp16, tag="I")     # [i_r; i_i]
            nc.scalar.activation(I[0:HALF, :], tA[0:HALF, :], Sigmoid,
                                 bias=bias_s[0:HALF, 0:1])
            nc.scalar.activation(I[HALF:P, :], tB[0:HALF, :], Sigmoid,
                                 bias=bias_s[0:HALF, 3:4])
            O = work.tile([P, cols], fp16, tag="O")     # [o_r; o_i]
            nc.scalar.activation(O[0:HALF, :], tA[HALF:P, :], Sigmoid,
                                 bias=bias_s[HALF:P, 4:5])
            nc.scalar.activation(O[HALF:P, :], tB[HALF:P, :], Sigmoid,
                                 bias=bias_s[HALF:P, 1:2])
            O2 = work.tile([P, cols], fp16, tag="O2")   # [o_i; o_r]
            nc.scalar.activation(O2[0:HALF, :], tB[HALF:P, :], Sigmoid,
                                 bias=bias_s[HALF:P, 1:2])
            nc.scalar.activation(O2[HALF:P, :], tA[HALF:P, :], Sigmoid,
                                 bias=bias_s[HALF:P, 4:5])

            # c gate direct: [re|im] packed, accumulate both components
            pt_c = ps_c.tile([P, cols], f32, tag="ptc")
            conv_block(pt_c, 3, True, False)
            conv_block(pt_c, 4, False, True)

            # CTs = [cti; -ctr] straight from PSUM via partition-crossed
            # activations, so no SBUF->SBUF swap DMAs are needed
            CT = work.tile([P, cols], fp16, tag="CT")      # [ctr; cti]
            nc.scalar.activation(CT[:], pt_c[:], Tanh, bias=bias_s[:, 2:3])
            CTs = work.tile([P, cols], fp16, tag="CTs")    # [cti; -ctr]
            nc.scalar.activation(CTs[0:HALF, :], pt_c[HALF:P, :], Tanh,
                                 bias=bias_s[HALF:P, 2:3])
            nc.scalar.activation(CTs[HALF:P, :], pt_c[0:HALF, :], Tanh,
                                 bias=bias_s[0:HALF, 5:6], scale=-1.0)

            # i (*) ct (complex): product halves written to base-0/base-64 so
            # every TensorTensor keeps same-base inputs
            P1 = work.tile([P, cols], fp16, tag="P1")      # [ir*ctr ; ir*cti]
            nc.vector.tensor_mul(P1[0:HALF, :], I[0:HALF, :], CT[0:HALF, :])
            nc.vector.tensor_mul(P1[HALF:P, :], I[0:HALF, :], CTs[0:HALF, :])
            P2 = work.tile([P, cols], fp16, tag="P2")      # [ii*cti ; -ii*ctr]
            nc.vector.tensor_mul(P2[0:HALF, :], I[HALF:P, :], CT[HALF:P, :])
            nc.vector.tensor_mul(P2[HALF:P, :], I[HALF:P, :], CTs[HALF:P, :])
            tmp = work.tile([P, cols], fp16, tag="tmp")
            nc.vector.tensor_sub(tmp[:], P1[:], P2[:])

            xc_t = work.tile([P, cols], fp16, tag="xc_t")
            nc.sync.dma_start(xc_t[:], xc_d[:, b, r0:r0 + nrows, :])
            cnew = work.tile([P, cols], fp16, tag="cnew")
            nc.vector.tensor_add(cnew[:], xc_t[:], tmp[:])
            nc.sync.dma_start(c_d[:, b, r0:r0 + nrows, :], cnew[:])

            T = work.tile([P, cols], fp16, tag="T")        # [tr; ti]
            nc.scalar.activation(T[:], cnew[:], Tanh)
            Tn = work.tile([P, cols], fp16, tag="Tn")      # [.. ; -ti]
            nc.scalar.activation(Tn[HALF:P, :], cnew[HALF:P, :], Tanh,
                                 scale=-1.0)

            Q1 = work.tile([P, cols], fp16, tag="Q1")      # [or*tr ; oi*tr]
            nc.vector.tensor_mul(Q1[0:HALF, :], O[0:HALF, :], T[0:HALF, :])
            nc.vector.tensor_mul(Q1[HALF:P, :], O2[0:HALF, :], T[0:HALF, :])
            Q2 = work.tile([P, cols], fp16, tag="Q2")      # [oi*ti ; -or*ti]
            nc.vector.tensor_mul(Q2[0:HALF, :], O[HALF:P, :], T[HALF:P, :])
            nc.vector.tensor_mul(Q2[HALF:P, :], O2[HALF:P, :], Tn[HALF:P, :])

            hnew = work.tile([P, cols], fp16, tag="hnew")
            nc.vector.tensor_sub(hnew[:], Q1[:], Q2[:])
            nc.sync.dma_start(h_d[:, b, r0:r0 + nrows, :], hnew[:])
            return last_mm[0]

        # first tile small (PE starts on less DMA'd data), last tiles small
        # (short post-matmul epilogue chain); z-chunk loads two tiles ahead
        SCHEDULE = [(0, 0, 8), (0, 8, 8), (0, 16, 16), (0, 32, 16),
                    (0, 48, 16), (1, 0, 16), (1, 16, 16), (1, 32, 16),
                    (1, 48, 8), (1, 56, 8)]
        deferred = {1: [(0, 'B')], 2: [(0, 'C')], 3: [(1, 'A')],
                    4: [(1, 'B')], 5: [(1, 'C')]}
        anchor = None
        for tidx, (b, r0, nrows) in enumerate(SCHEDULE):
            for b2, ch2 in deferred.pop(tidx, []):
                load_z(b2, ch2, after=anchor)
            anchor = macro_tile(b, r0, nrows)

    _split_excess_waits(nc)
    return nc


def _prep_inputs(inputs):
    """Host-side shard + layout prep. Returns per-core in_maps."""
    f16 = np.float16
    x = np.asarray(inputs['x'], np.float32)
    h_prev = np.asarray(inputs['h_prev'], np.float32)
    c_prev = np.asarray(inputs['c_prev'], np.float32)

    xr, xi = x[:, :HALF], x[:, HALF:]
    hr, hi = h_prev[:, :HALF], h_prev[:, HALF:]
    cr, ci = c_prev[:, :HALF], c_prev[:, HALF:]

    # combined conv input, channel-major, zero-padded, fp16: [128, B, 66, 66]
    def prep_z(a):
        z = a.transpose(1, 0, 2, 3)
        return np.pad(z, ((0, 0), (0, 0), (1, 1), (1, 1))).astype(f16)
    zr_f = np.concatenate([xr, hr], axis=1)
    zi_f = np.concatenate([xi, hi], axis=1)
    zr = prep_z(zr_f)
    zi = prep_z(zi_f)
    zs = prep_z(zr_f + zi_f)

    # x (*) c_prev (complex elementwise), channel-major fp16: [128, B, 64, 64]
    xc = np.concatenate([xr * cr - xi * ci, xr * ci + xi * cr],
                        axis=1).transpose(1, 0, 2, 3).astype(f16)

    # packed gate weights: [cin 128, 45, cout 128] fp16.
    # blocks of 9 taps: io-gauss t1 (Wr), t2 (Wi-Wr), t3 (Wr+Wi) with halves
    # [i | o]; then c-direct [Wr_c | Wi_c] on zr and [-Wi_c | Wr_c] on zi.
    Wg = {}
    for gn in ('i', 'o', 'c'):
        Wg['r' + gn] = np.asarray(inputs['Wr_' + gn], np.float32)  # [64,128,3,3]
        Wg['i' + gn] = np.asarray(inputs['Wi_' + gn], np.float32)
    wts = np.empty((NM, P, P), np.float32)
    for t in range(9):
        kh, kw = t // 3, t % 3
        for blk, (li, lo) in enumerate((
                (Wg['ri'], Wg['ro']),                          # t1: Wr
                (Wg['ii'] - Wg['ri'], Wg['io'] - Wg['ro']),    # t2: Wi-Wr
                (Wg['ri'] + Wg['ii'], Wg['ro'] + Wg['io']),    # t3: Wr+Wi
                (Wg['rc'], Wg['ic']),                          # c on zr
                (-Wg['ic'], Wg['rc']))):                       # c on zi
            wts[blk * 9 + t, :, :HALF] = li[:, :, kh, kw].T
            wts[blk * 9 + t, :, HALF:] = lo[:, :, kh, kw].T
    wts = np.ascontiguousarray(wts.transpose(1, 0, 2)).astype(f16)

    # bias columns: 0:[br_i;bi_i] 1:[br_o;bi_o] 2:[br_c;bi_c]
    #               3:[bi_i;br_i] 4:[bi_o;br_o] 5:-col2
    bias = np.empty((P, 6), np.float32)
    for g, gn in enumerate('ioc'):
        br = np.asarray(inputs['br_' + gn], np.float32)
        bi = np.asarray(inputs['bi_' + gn], np.float32)
        bias[:, g] = np.concatenate([br, bi])
        if gn != 'c':
            bias[:, 3 + g] = np.concatenate([bi, br])
    bias[:, 5] = -bias[:, 2]

    in_maps = []
    for c in range(N_CORES):
        sl = slice(c * B_CORE, (c + 1) * B_CORE)
        in_maps.append({
            "zs": np.ascontiguousarray(zs[:, sl]),
            "zr": np.ascontiguousarray(zr[:, sl]),
            "zi": np.ascontiguousarray(zi[:, sl]),
            "wts": wts,
            "bias": bias,
            "xc": np.ascontiguousarray(xc[:, sl]),
        })
    return in_maps


def _gather_outputs(results):
    h_full = np.empty((B, P, H, W), np.float32)
    c_full = np.empty((B, P, H, W), np.float32)
    for c in range(N_CORES):
        sl = slice(c * B_CORE, (c + 1) * B_CORE)
        h_full[sl] = results[c]["h_out"].transpose(1, 0, 2, 3).astype(np.float32)
        c_full[sl] = results[c]["c_out"].transpose(1, 0, 2, 3).astype(np.float32)
    return h_full, c_full


def _run(inputs, trace=False, trace_kwargs=None):
    from concourse.bass_utils import run_bass_kernel_spmd

    if "nc" not in _CACHE:
        _CACHE["nc"] = _build_program()
    nc = _CACHE["nc"]
    in_maps = _prep_inputs(inputs)
    r = run_bass_kernel_spmd(nc, in_maps, list(range(N_CORES)),
                             trace=trace, trace_kwargs=trace_kwargs or {})
    return _gather_outputs(r.results), r


def kernel(**inputs):
    (h_full, c_full), _ = _run(inputs)
    return h_full, c_full


# revision 15
# speedup vs baseline: 1.1424x; 1.1424x over previous
"""ConvLSTM cell (complex-valued gates) on 8 TRN2 NeuronCores.

Strategy
--------
Data-parallel over batch: 16 images -> 2 per core. Per core, the three
live gates' complex 3x3 convs are computed as shifted matmuls
accumulated in PSUM:

    out[128, 512] += lhsT[128in, 128out].T @ z_shift[128in, 512]

The i and o gates (both sigmoid) are computed with the Gauss 3-mult
complex trick, packed pairwise so every pass keeps the full 128-wide
PE output:

    t1 = conv(zr+zi, Wr)        t2 = conv(zr, Wi-Wr)   t3 = conv(zi, Wr+Wi)
    y_r = t1 - t3               y_i = t1 + t2

with lhsT halves [t_i | t_o]. That is 3 passes per tap for both gates
vs 4 direct. The c gate stays direct (2 passes/tap, [re|im] packed).
Total 45 PE passes/tap-set vs 54 direct: ~154 us tensor-engine floor.

The Gauss combines are folded into PSUM accumulation so the epilogue
stays as cheap as the direct version (extra DVE/ScalarE traffic was
measured to back-pressure the PE stream): t1 accumulates in T1, one
ScalarE copy duplicates it to R3, then t2 accumulates on top of T1
(-> y_i) and -t3 (host-negated weights) on top of R3 (-> y_r), with
the c-gate's zr passes filling the PE while the copy drains.

All matmul operands are fp16 (full PE speed). ScalarE applies
sigmoid/tanh from PSUM/SBUF with the per-channel bias fused. VectorE
does the Gauss combines (PSUM->SBUF) and the complex elementwise update
in fp16. x (*) c_prev is an input-only elementwise term precomputed on
the host and added on-chip. Outputs leave as fp16, upcast on host.

The spatial dim is processed in 10 macro-tiles per core (8..16 rows x
1024 cols max), 4 PSUM accumulation tiles (t1/t2/t3/c) per chunk.
z is kept resident in SBUF, zero-padded to 66x66 on the host so conv
taps are plain shifted access patterns. Weight DMA is split into
per-stream blocks in first-use order so the first matmul starts early.
"""
import sys
import numpy as np

sys.path.insert(0, "/opt/trn_rl_repo")

P = 128          # partitions / channels (64 real + 64 imag)
HALF = 64
B = 16           # full batch
N_CORES = 8
B_CORE = B // N_CORES   # batch per core
H = W = 64
HP = WP = 66     # padded spatial
MACRO = 16 * W   # max columns per macro tile
NM = 45          # packed weight passes: 9 taps x (3 io-gauss + 2 c-direct)

_CACHE = {}


def _apply_drain_patch(tile_mod):
    """The kernel-tail drain aggregates one wait per live proc-semaphore, but
    walrus rejects instructions with more than a few sync waits. Split the
    tail waits across a chain of single-wait drains."""
    if getattr(tile_mod.TileContext, "_drain_patched", False):
        return

    def _patched(self, tick_clock, wait_clock):
        ScopedClock = tile_mod.ScopedClock
        nc = self.nc
        drain_inst = nc.sync.drain()
        wait_clock.add_sem_waits(
            drain_inst.ins, ScopedClock({None: tick_clock.global_clock})
        )
        NW = 3
        si = drain_inst.ins.sync_info
        if si is not None and si.on_wait and len(si.on_wait) > NW:
            conds = list(si.on_wait)
            si.on_wait = conds[:NW]
            rest = conds[NW:]
            while rest:
                extra = nc.sync.drain()
                esi = extra.ins.sync_info
                if esi is None:
                    import bass_rust
                    extra.ins.sync_info = bass_rust.SyncInfo(
                        on_wait=rest[:NW], on_update=[])
                else:
                    esi.on_wait = rest[:NW]
                rest = rest[NW:]

        nc.all_engine_barrier()
        assert self.sems is not None
        popped = nc._tile_sem_poison_stack.pop()
        assert popped is self._sem_poison
        nc.clear_and_free_semaphores(list(self.sems.allocated().values()))
        nc.all_engine_barrier()

    tile_mod.TileContext._drain_and_barrier = _patched
    tile_mod.TileContext._drain_patched = True


def _split_excess_waits(nc, max_waits=1):
    """walrus's per-instruction sync-wait slots are tight (1 for some ISA
    structs). Hoist excess waits into same-engine no-ops inserted directly
    before the instruction — identical semantics, per-engine order kept."""
    import concourse.mybir as mybir
    n_new = 0
    for fn in nc.m.functions:
        for bb in fn.blocks:
            il = bb.instructions
            out = []
            for inst in il:
                si = inst.sync_info
                if si is not None and si.on_wait and len(si.on_wait) > max_waits:
                    conds = list(si.on_wait)
                    si.on_wait = conds[:max_waits]
                    rest = conds[max_waits:]
                    for j in range(0, len(rest), max_waits):
                        nop = mybir.InstNoOp(
                            name=f"{inst.name}_w{j}",
                            sync_info=mybir.SyncInfo(
                                on_wait=rest[j:j + max_waits], on_update=[]),
                            bass_nofuse=True,
                            engine=inst.engine,
                        )
                        out.append(nop)
                        n_new += 1
                out.append(inst)
            if n_new:
                il[:] = out
    return n_new


def _build_program():
    import concourse.bass as bass
    import concourse.tile as tile
    from concourse import mybir
    from contextlib import ExitStack

    _apply_drain_patch(tile)
    fp16 = mybir.dt.float16
    f32 = mybir.dt.float32
    Sigmoid = mybir.ActivationFunctionType.Sigmoid
    Tanh = mybir.ActivationFunctionType.Tanh
    Copy = mybir.ActivationFunctionType.Copy

    nc = bass.Bass("TRN2", target_bir_lowering=False, debug=False)
    zs_d = nc.dram_tensor("zs", [P, B_CORE, HP, WP], fp16, kind="ExternalInput").ap()
    zr_d = nc.dram_tensor("zr", [P, B_CORE, HP, WP], fp16, kind="ExternalInput").ap()
    zi_d = nc.dram_tensor("zi", [P, B_CORE, HP, WP], fp16, kind="ExternalInput").ap()
    w_d = nc.dram_tensor("wts", [P, NM, P], fp16, kind="ExternalInput").ap()
    b_d = nc.dram_tensor("bias", [P, 6], f32, kind="ExternalInput").ap()
    xc_d = nc.dram_tensor("xc", [P, B_CORE, H, W], fp16, kind="ExternalInput").ap()
    id_d = nc.dram_tensor("ident", [P, P], fp16, kind="ExternalInput").ap()
    h_d = nc.dram_tensor("h_out", [P, B_CORE, H, W], fp16, kind="ExternalOutput").ap()
    c_d = nc.dram_tensor("c_out", [P, B_CORE, H, W], fp16, kind="ExternalOutput").ap()

    # padded-row chunks (overlapping): A=[0:18) B=[16:34) C=[32:66)
    Z_CHUNKS = {'A': (0, 18), 'B': (16, 18), 'C': (32, 34)}
    # matmul streams: (z component, weight-block offset). io-gauss t1/t2/t3
    # then c-direct zr/zi; per block 9 taps.
    STREAMS = [('s', 0), ('r', 9), ('i', 18), ('r', 27), ('i', 36)]

    with tile.TileContext(nc) as tc, ExitStack() as ctx:
        const = ctx.enter_context(tc.tile_pool(name="const", bufs=1))
        w_s = const.tile([P, NM, P], fp16, name="wts")
        z_ch = {}

        def load_w(blk, after=None):
            dm = nc.sync.dma_start(w_s[:, blk * 9:(blk + 1) * 9, :],
                                   w_d[:, blk * 9:(blk + 1) * 9, :])
            if after is not None:
                tile.add_dep_helper(dm.ins, after, reason="defer w block")

        def load_z(b, ch, comps='sri', after=None):
            row0, nr = Z_CHUNKS[ch]
            for comp in comps:
                zt_d = {'s': zs_d, 'r': zr_d, 'i': zi_d}[comp]
                t = const.tile([P, nr, WP], fp16, name=f"z{comp}_{b}_{ch}")
                dm = nc.sync.dma_start(t[:], zt_d[:, b, row0:row0 + nr, :])
                if after is not None:
                    # hold the transfer back until the anchor matmul retires so
                    # it can't steal HBM bandwidth from earlier-needed loads
                    tile.add_dep_helper(dm.ins, after,
                                        reason="defer non-critical z load")
                z_ch[(comp, b, ch)] = t

        # upfront loads in first-consumption order so the first macro-tile's
        # matmuls start after ~0.6MB of DMA instead of ~4MB
        load_w(0)
        load_z(0, 'A', comps='s')
        load_z(0, 'A', comps='r')
        load_w(3)
        ident_s = const.tile([P, P], fp16, name="ident")
        nc.sync.dma_start(ident_s[:], id_d[:])
        load_z(0, 'A', comps='i')
        load_w(2)
        load_w(1)
        load_w(4)
        bias_s = const.tile([P, 6], f32)
        nc.sync.dma_start(bias_s[:], b_d[:])

        ps_1 = ctx.enter_context(tc.tile_pool(name="ps_1", bufs=1, space="PSUM"))
        ps_3 = ctx.enter_context(tc.tile_pool(name="ps_3", bufs=1, space="PSUM"))
        ps_c = ctx.enter_context(tc.tile_pool(name="ps_c", bufs=1, space="PSUM"))
        work = ctx.enter_context(tc.tile_pool(name="work", bufs=2))

        def macro_tile(b, r0, nrows):
            cols = nrows * W
            if r0 + nrows + 1 < 18:
                ch = 'A'
            elif r0 >= 16 and r0 + nrows + 1 < 34:
                ch = 'B'
            else:
                ch = 'C'
            roff = Z_CHUNKS[ch][0]   # chunk's first padded row

            last_mm = [None]

            def conv_block(pt, si, start, stop, skip=False):
                comp, mbase = STREAMS[si]
                z_s = z_ch[(comp, b, ch)]
                for t in range(9):
                    kh, kw = t // 3, t % 3
                    for half in range(nrows // 8):
                        r0h = r0 + half * 8 - roff
                        mm = nc.tensor.matmul(
                            pt[:, half * 512:(half + 1) * 512],
                            w_s[:, mbase + t, :],
                            z_s[:, r0h + kh:r0h + kh + 8, kw:kw + 64],
                            start=(start and t == 0), stop=(stop and t == 8),
                            skip_group_check=skip,
                        )
                        last_mm[0] = mm.ins
                return pt

            # io gauss unit, combines folded into PSUM accumulation:
            #   T1 = t1 (+ t2 later) -> y_i
            #   R3 = ident @ fp16(t1), then + -t3 -> y_r
            # Both PSUM groups are conventional (one start, one stop); the
            # t1 staging copy runs on the idle GpSimd engine and the c-gate
            # zr passes fill the PE while it drains.
            T1 = ps_1.tile([P, cols], f32, tag="pt1")   # -> [y_i_i | y_i_o]
            R3 = ps_3.tile([P, cols], f32, tag="pt3")   # -> [y_r_i | y_r_o]
            pt_c = ps_c.tile([P, cols], f32, tag="ptc")
            conv_block(T1, 0, True, False)
            T1s = work.tile([P, cols], fp16, tag="T1s")
            nc.scalar.activation(T1s[:], T1[:], Copy)
            conv_block(pt_c, 3, True, False)
            for half in range(nrows // 8):
                nc.tensor.matmul(
                    R3[:, half * 512:(half + 1) * 512], ident_s[:],
                    T1s[:, half * 512:(half + 1) * 512],
                    start=True, stop=False)
            conv_block(R3, 2, False, True, skip=True)
            conv_block(T1, 1, False, True, skip=True)
            conv_block(pt_c, 4, False, True)

            # sigmoids with fused bias; ScalarE may cross partition bases
            I = work.tile([P, cols], fp16, tag="I")     # [i_r; i_i]
            nc.scalar.activation(I[0:HALF, :], R3[0:HALF, :], Sigmoid,
                                 bias=bias_s[0:HALF, 0:1])
            nc.scalar.activation(I[HALF:P, :], T1[0:HALF, :], Sigmoid,
                                 bias=bias_s[0:HALF, 3:4])
            O = work.tile([P, cols], fp16, tag="O")     # [o_r; o_i]
            nc.scalar.activation(O[0:HALF, :], R3[HALF:P, :], Sigmoid,
                                 bias=bias_s[HALF:P, 4:5])
            nc.scalar.activation(O[HALF:P, :], T1[HALF:P, :], Sigmoid,
                                 bias=bias_s[HALF:P, 1:2])
            O2 = work.tile([P, cols], fp16, tag="O2")   # [o_i; o_r]
            nc.scalar.activation(O2[0:HALF, :], T1[HALF:P, :], Sigmoid,
                                 bias=bias_s[HALF:P, 1:2])
            nc.scalar.activation(O2[HALF:P, :], R3[HALF:P, :], Sigmoid,
                                 bias=bias_s[HALF:P, 4:5])

            # CTs = [cti; -ctr] straight from PSUM via partition-crossed
            # activations, so no SBUF->SBUF swap DMAs are needed
            CT = work.tile([P, cols], fp16, tag="CT")      # [ctr; cti]
            nc.scalar.activation(CT[:], pt_c[:], Tanh, bias=bias_s[:, 2:3])
            CTs = work.tile([P, cols], fp16, tag="CTs")    # [cti; -ctr]
            nc.scalar.activation(CTs[0:HALF, :], pt_c[HALF:P, :], Tanh,
                                 bias=bias_s[HALF:P, 2:3])
            nc.scalar.activation(CTs[HALF:P, :], pt_c[0:HALF, :], Tanh,
                                 bias=bias_s[0:HALF, 5:6], scale=-1.0)

            # i (*) ct (complex): product halves written to base-0/base-64 so
            # every TensorTensor keeps same-base inputs
            P1 = work.tile([P, cols], fp16, tag="P1")      # [ir*ctr ; ir*cti]
            nc.vector.tensor_mul(P1[0:HALF, :], I[0:HALF, :], CT[0:HALF, :])
            nc.vector.tensor_mul(P1[HALF:P, :], I[0:HALF, :], CTs[0:HALF, :])
            P2 = work.tile([P, cols], fp16, tag="P2")      # [ii*cti ; -ii*ctr]
            nc.vector.tensor_mul(P2[0:HALF, :], I[HALF:P, :], CT[HALF:P, :])
            nc.vector.tensor_mul(P2[HALF:P, :], I[HALF:P, :], CTs[HALF:P, :])
            tmp = work.tile([P, cols], fp16, tag="tmp")
            nc.vector.tensor_sub(tmp[:], P1[:], P2[:])

            xc_t = work.tile([P, cols], fp16, tag="xc_t")
            nc.sync.dma_start(xc_t[:], xc_d[:, b, r0:r0 + nrows, :])
            cnew = work.tile([P, cols], fp16, tag="cnew")
            nc.vector.tensor_add(cnew[:], xc_t[:], tmp[:])
            nc.sync.dma_start(c_d[:, b, r0:r0 + nrows, :], cnew[:])

            T = work.tile([P, cols], fp16, tag="T")        # [tr; ti]
            nc.scalar.activation(T[:], cnew[:], Tanh)
            Tn = work.tile([P, cols], fp16, tag="Tn")      # [.. ; -ti]
            nc.scalar.activation(Tn[HALF:P, :], cnew[HALF:P, :], Tanh,
                                 scale=-1.0)

            Q1 = work.tile([P, cols], fp16, tag="Q1")      # [or*tr ; oi*tr]
            nc.vector.tensor_mul(Q1[0:HALF, :], O[0:HALF, :], T[0:HALF, :])
            nc.vector.tensor_mul(Q1[HALF:P, :], O2[0:HALF, :], T[0:HALF, :])
            Q2 = work.tile([P, cols], fp16, tag="Q2")      # [oi*ti ; -or*ti]
            nc.vector.tensor_mul(Q2[0:HALF, :], O[HALF:P, :], T[HALF:P, :])
            nc.vector.tensor_mul(Q2[HALF:P, :], O2[HALF:P, :], Tn[HALF:P, :])

            hnew = work.tile([P, cols], fp16, tag="hnew")
            nc.vector.tensor_sub(hnew[:], Q1[:], Q2[:])
            nc.sync.dma_start(h_d[:, b, r0:r0 + nrows, :], hnew[:])
            return last_mm[0]

        # first tile small (PE starts on less DMA'd data), last tiles small
        # (short post-matmul epilogue chain); z-chunk loads two tiles ahead
        SCHEDULE = [(0, 0, 8), (0, 8, 8), (0, 16, 16), (0, 32, 16),
                    (0, 48, 16), (1, 0, 16), (1, 16, 16), (1, 32, 16),
                    (1, 48, 8), (1, 56, 8)]
        deferred = {1: [(0, 'B')], 2: [(0, 'C')], 3: [(1, 'A')],
                    4: [(1, 'B')], 5: [(1, 'C')]}
        anchor = None
        for tidx, (b, r0, nrows) in enumerate(SCHEDULE):
            for b2, ch2 in deferred.pop(tidx, []):
                load_z(b2, ch2, after=anchor)
            anchor = macro_tile(b, r0, nrows)

    _split_excess_waits(nc)
    return nc


def _prep_inputs(inputs):
    """Host-side shard + layout prep. Returns per-core in_maps."""
    f16 = np.float16
    x = np.asarray(inputs['x'], np.float32)
    h_prev = np.asarray(inputs['h_prev'], np.float32)
    c_prev = np.asarray(inputs['c_prev'], np.float32)

    xr, xi = x[:, :HALF], x[:, HALF:]
    hr, hi = h_prev[:, :HALF], h_prev[:, HALF:]
    cr, ci = c_prev[:, :HALF], c_prev[:, HALF:]

    # combined conv input, channel-major, zero-padded, fp16: [128, B, 66, 66]
    def prep_z(a):
        z = a.transpose(1, 0, 2, 3)
        return np.pad(z, ((0, 0), (0, 0), (1, 1), (1, 1))).astype(f16)
    zr_f = np.concatenate([xr, hr], axis=1)
    zi_f = np.concatenate([xi, hi], axis=1)
    zr = prep_z(zr_f)
    zi = prep_z(zi_f)
    zs = prep_z(zr_f + zi_f)

    # x (*) c_prev (complex elementwise), channel-major fp16: [128, B, 64, 64]
    xc = np.concatenate([xr * cr - xi * ci, xr * ci + xi * cr],
                        axis=1).transpose(1, 0, 2, 3).astype(f16)

    # packed gate weights: [cin 128, 45, cout 128] fp16.
    # blocks of 9 taps: io-gauss t1 (Wr), t2 (Wi-Wr), t3 (Wr+Wi) with halves
    # [i | o]; then c-direct [Wr_c | Wi_c] on zr and [-Wi_c | Wr_c] on zi.
    Wg = {}
    for gn in ('i', 'o', 'c'):
        Wg['r' + gn] = np.asarray(inputs['Wr_' + gn], np.float32)  # [64,128,3,3]
        Wg['i' + gn] = np.asarray(inputs['Wi_' + gn], np.float32)
    wts = np.empty((NM, P, P), np.float32)
    for t in range(9):
        kh, kw = t // 3, t % 3
        for blk, (li, lo) in enumerate((
                (Wg['ri'], Wg['ro']),                          # t1: Wr
                (Wg['ii'] - Wg['ri'], Wg['io'] - Wg['ro']),    # t2: Wi-Wr
                (-Wg['ri'] - Wg['ii'], -Wg['ro'] - Wg['io']),  # -t3: -(Wr+Wi)
                (Wg['rc'], Wg['ic']),                          # c on zr
                (-Wg['ic'], Wg['rc']))):                       # c on zi
            wts[blk * 9 + t, :, :HALF] = li[:, :, kh, kw].T
            wts[blk * 9 + t, :, HALF:] = lo[:, :, kh, kw].T
    wts = np.ascontiguousarray(wts.transpose(1, 0, 2)).astype(f16)

    # bias columns: 0:[br_i;bi_i] 1:[br_o;bi_o] 2:[br_c;bi_c]
    #               3:[bi_i;br_i] 4:[bi_o;br_o] 5:-col2
    bias = np.empty((P, 6), np.float32)
    for g, gn in enumerate('ioc'):
        br = np.asarray(inputs['br_' + gn], np.float32)
        bi = np.asarray(inputs['bi_' + gn], np.float32)
        bias[:, g] = np.concatenate([br, bi])
        if gn != 'c':
            bias[:, 3 + g] = np.concatenate([bi, br])
    bias[:, 5] = -bias[:, 2]

    in_maps = []
    for c in range(N_CORES):
        sl = slice(c * B_CORE, (c + 1) * B_CORE)
        in_maps.append({
            "zs": np.ascontiguousarray(zs[:, sl]),
            "zr": np.ascontiguousarray(zr[:, sl]),
            "zi": np.ascontiguousarray(zi[:, sl]),
            "wts": wts,
            "bias": bias,
            "xc": np.ascontiguousarray(xc[:, sl]),
            "ident": np.eye(P, dtype=f16),
        })
    return in_maps


def _gather_outputs(results):
    h_full = np.empty((B, P, H, W), np.float32)
    c_full = np.empty((B, P, H, W), np.float32)
    for c in range(N_CORES):
        sl = slice(c * B_CORE, (c + 1) * B_CORE)
        h_full[sl] = results[c]["h_out"].transpose(1, 0, 2, 3).astype(np.float32)
        c_full[sl] = results[c]["c_out"].transpose(1, 0, 2, 3).astype(np.float32)
    return h_full, c_full


def _run(inputs, trace=False, trace_kwargs=None):
    from concourse.bass_utils import run_bass_kernel_spmd

    if "nc" not in _CACHE:
        _CACHE["nc"] = _build_program()
    nc = _CACHE["nc"]
    in_maps = _prep_inputs(inputs)
    r = run_bass_kernel_spmd(nc, in_maps, list(range(N_CORES)),
                             trace=trace, trace_kwargs=trace_kwargs or {})
    return _gather_outputs(r.results), r


def kernel(**inputs):
    (h_full, c_full), _ = _run(inputs)
    return h_full, c_full


# revision 19
# speedup vs baseline: 1.1730x; 1.0268x over previous
"""ConvLSTM cell (complex-valued gates) on 8 TRN2 NeuronCores.

Strategy
--------
Data-parallel over batch: 16 images -> 2 per core. Per core, the three
live gates' complex 3x3 convs are computed as shifted matmuls
accumulated in PSUM:

    out[128, 512] += lhsT[128in, 128out].T @ z_shift[128in, 512]

The i and o gates (both sigmoid) are computed with the Gauss 3-mult
complex trick, packed pairwise so every pass keeps the full 128-wide
PE output:

    t1 = conv(zr+zi, Wr)        t2 = conv(zr, Wi-Wr)   t3 = conv(zi, Wr+Wi)
    y_r = t1 - t3               y_i = t1 + t2

with lhsT halves [t_i | t_o]. That is 3 passes per tap for both gates
vs 4 direct. The c gate stays direct (2 passes/tap, [re|im] packed).
Total 45 PE passes/tap-set vs 54 direct: ~154 us tensor-engine floor.

The Gauss combines are folded into PSUM accumulation so the epilogue
stays as cheap as the direct version (extra DVE/ScalarE traffic was
measured to back-pressure the PE stream): t1 accumulates in T1, one
ScalarE copy duplicates it to R3, then t2 accumulates on top of T1
(-> y_i) and -t3 (host-negated weights) on top of R3 (-> y_r), with
the c-gate's zr passes filling the PE while the copy drains.

All matmul operands are fp16 (full PE speed). ScalarE applies
sigmoid/tanh from PSUM/SBUF with the per-channel bias fused. VectorE
does the Gauss combines (PSUM->SBUF) and the complex elementwise update
in fp16. x (*) c_prev is an input-only elementwise term precomputed on
the host and added on-chip. Outputs leave as fp16, upcast on host.

The spatial dim is processed in 10 macro-tiles per core (8..16 rows x
1024 cols max), 4 PSUM accumulation tiles (t1/t2/t3/c) per chunk.
z is kept resident in SBUF, zero-padded to 66x66 on the host so conv
taps are plain shifted access patterns. Weight DMA is split into
per-stream blocks in first-use order so the first matmul starts early.
"""
import sys
import numpy as np

sys.path.insert(0, "/opt/trn_rl_repo")

P = 128          # partitions / channels (64 real + 64 imag)
HALF = 64
B = 16           # full batch
N_CORES = 8
B_CORE = B // N_CORES   # batch per core
H = W = 64
HP = WP = 66     # padded spatial
MACRO = 16 * W   # max columns per macro tile
NM = 45          # packed weight passes: 9 taps x (3 io-gauss + 2 c-direct)

_CACHE = {}


def _apply_drain_patch(tile_mod):
    """The kernel-tail drain aggregates one wait per live proc-semaphore, but
    walrus rejects instructions with more than a few sync waits. Split the
    tail waits across a chain of single-wait drains."""
    if getattr(tile_mod.TileContext, "_drain_patched", False):
        return

    def _patched(self, tick_clock, wait_clock):
        ScopedClock = tile_mod.ScopedClock
        nc = self.nc
        drain_inst = nc.sync.drain()
        wait_clock.add_sem_waits(
            drain_inst.ins, ScopedClock({None: tick_clock.global_clock})
        )
        NW = 3
        si = drain_inst.ins.sync_info
        if si is not None and si.on_wait and len(si.on_wait) > NW:
            conds = list(si.on_wait)
            si.on_wait = conds[:NW]
            rest = conds[NW:]
            while rest:
                extra = nc.sync.drain()
                esi = extra.ins.sync_info
                if esi is None:
                    import bass_rust
                    extra.ins.sync_info = bass_rust.SyncInfo(
                        on_wait=rest[:NW], on_update=[])
                else:
                    esi.on_wait = rest[:NW]
                rest = rest[NW:]

        nc.all_engine_barrier()
        assert self.sems is not None
        popped = nc._tile_sem_poison_stack.pop()
        assert popped is self._sem_poison
        nc.clear_and_free_semaphores(list(self.sems.allocated().values()))
        nc.all_engine_barrier()

    tile_mod.TileContext._drain_and_barrier = _patched
    tile_mod.TileContext._drain_patched = True


def _split_excess_waits(nc, max_waits=1):
    """walrus's per-instruction sync-wait slots are tight (1 for some ISA
    structs). Hoist excess waits into same-engine no-ops inserted directly
    before the instruction — identical semantics, per-engine order kept."""
    import concourse.mybir as mybir
    n_new = 0
    for fn in nc.m.functions:
        for bb in fn.blocks:
            il = bb.instructions
            out = []
            for inst in il:
                si = inst.sync_info
                if si is not None and si.on_wait and len(si.on_wait) > max_waits:
                    conds = list(si.on_wait)
                    si.on_wait = conds[:max_waits]
                    rest = conds[max_waits:]
                    for j in range(0, len(rest), max_waits):
                        nop = mybir.InstNoOp(
                            name=f"{inst.name}_w{j}",
                            sync_info=mybir.SyncInfo(
                                on_wait=rest[j:j + max_waits], on_update=[]),
                            bass_nofuse=True,
                            engine=inst.engine,
                        )
                        out.append(nop)
                        n_new += 1
                out.append(inst)
            if n_new:
                il[:] = out
    return n_new


def _build_program():
    import concourse.bass as bass
    import concourse.tile as tile
    from concourse import mybir
    from contextlib import ExitStack

    _apply_drain_patch(tile)
    fp16 = mybir.dt.float16
    f32 = mybir.dt.float32
    Sigmoid = mybir.ActivationFunctionType.Sigmoid
    Tanh = mybir.ActivationFunctionType.Tanh
    Copy = mybir.ActivationFunctionType.Copy

    nc = bass.Bass("TRN2", target_bir_lowering=False, debug=False)
    zs_d = nc.dram_tensor("zs", [P, B_CORE, HP, WP], fp16, kind="ExternalInput").ap()
    zr_d = nc.dram_tensor("zr", [P, B_CORE, HP, WP], fp16, kind="ExternalInput").ap()
    zi_d = nc.dram_tensor("zi", [P, B_CORE, HP, WP], fp16, kind="ExternalInput").ap()
    w_d = nc.dram_tensor("wts", [P, NM, P], fp16, kind="ExternalInput").ap()
    b_d = nc.dram_tensor("bias", [P, 6], f32, kind="ExternalInput").ap()
    xc_d = nc.dram_tensor("xc", [P, B_CORE, H, W], fp16, kind="ExternalInput").ap()
    id_d = nc.dram_tensor("ident", [P, P], fp16, kind="ExternalInput").ap()
    h_d = nc.dram_tensor("h_out", [P, B_CORE, H, W], fp16, kind="ExternalOutput").ap()
    c_d = nc.dram_tensor("c_out", [P, B_CORE, H, W], fp16, kind="ExternalOutput").ap()

    # padded-row chunks (overlapping): A=[0:18) B=[16:34) C=[32:66)
    Z_CHUNKS = {'A': (0, 18), 'B': (16, 18), 'C': (32, 34)}
    # matmul streams: (z component, weight-block offset). io-gauss t1/t2/t3
    # then c-direct zr/zi; per block 9 taps.
    STREAMS = [('s', 0), ('r', 9), ('i', 18), ('r', 27), ('i', 36)]

    with tile.TileContext(nc) as tc, ExitStack() as ctx:
        const = ctx.enter_context(tc.tile_pool(name="const", bufs=1))
        w_s = const.tile([P, NM, P], fp16, name="wts")
        z_ch = {}

        def load_w(blk, after=None):
            dm = nc.sync.dma_start(w_s[:, blk * 9:(blk + 1) * 9, :],
                                   w_d[:, blk * 9:(blk + 1) * 9, :])
            if after is not None:
                tile.add_dep_helper(dm.ins, after, reason="defer w block")

        def load_z(b, ch, comps='sri', after=None):
            row0, nr = Z_CHUNKS[ch]
            for comp in comps:
                zt_d = {'s': zs_d, 'r': zr_d, 'i': zi_d}[comp]
                t = const.tile([P, nr, WP], fp16, name=f"z{comp}_{b}_{ch}")
                dm = nc.sync.dma_start(t[:], zt_d[:, b, row0:row0 + nr, :])
                if after is not None:
                    # hold the transfer back until the anchor matmul retires so
                    # it can't steal HBM bandwidth from earlier-needed loads
                    tile.add_dep_helper(dm.ins, after,
                                        reason="defer non-critical z load")
                z_ch[(comp, b, ch)] = t

        # upfront loads in first-consumption order so the first macro-tile's
        # matmuls start after ~0.6MB of DMA instead of ~4MB
        load_w(0)
        load_z(0, 'A', comps='s')
        load_z(0, 'A', comps='r')
        load_w(3)
        ident_s = const.tile([P, P], fp16, name="ident")
        nc.sync.dma_start(ident_s[:], id_d[:])
        load_z(0, 'A', comps='i')
        load_w(2)
        load_w(1)
        load_w(4)
        bias_s = const.tile([P, 6], f32)
        nc.sync.dma_start(bias_s[:], b_d[:])

        ps_1 = ctx.enter_context(tc.tile_pool(name="ps_1", bufs=2, space="PSUM"))
        ps_3 = ctx.enter_context(tc.tile_pool(name="ps_3", bufs=2, space="PSUM"))
        ps_c = ctx.enter_context(tc.tile_pool(name="ps_c", bufs=2, space="PSUM"))
        work = ctx.enter_context(tc.tile_pool(name="work", bufs=2))

        def macro_tile(b, r0, nrows):
            cols = nrows * W
            if r0 + nrows + 1 < 18:
                ch = 'A'
            elif r0 >= 16 and r0 + nrows + 1 < 34:
                ch = 'B'
            else:
                ch = 'C'
            roff = Z_CHUNKS[ch][0]   # chunk's first padded row

            last_mm = [None]

            subs = ([(i * 8, 8) for i in range(nrows // 8)]
                    if nrows >= 8 else [(0, nrows)])

            def conv_block(pt, si, start, stop, skip=False):
                comp, mbase = STREAMS[si]
                z_s = z_ch[(comp, b, ch)]
                for t in range(9):
                    kh, kw = t // 3, t % 3
                    for rsub, nr in subs:
                        r0h = r0 + rsub - roff
                        mm = nc.tensor.matmul(
                            pt[:, rsub * W:(rsub + nr) * W],
                            w_s[:, mbase + t, :],
                            z_s[:, r0h + kh:r0h + kh + nr, kw:kw + 64],
                            start=(start and t == 0), stop=(stop and t == 8),
                            skip_group_check=skip,
                        )
                        last_mm[0] = mm.ins
                return pt

            # io gauss unit, combines folded into PSUM accumulation:
            #   T1 = t1 (+ t2 later) -> y_i
            #   R3 = ident @ fp16(t1), then + -t3 -> y_r
            # Both PSUM groups are conventional (one start, one stop); the
            # t1 staging copy runs on the idle GpSimd engine and the c-gate
            # zr passes fill the PE while it drains.
            T1 = ps_1.tile([P, cols], f32, tag="pt1")   # -> [y_i_i | y_i_o]
            R3 = ps_3.tile([P, cols], f32, tag="pt3")   # -> [y_r_i | y_r_o]
            pt_c = ps_c.tile([P, cols], f32, tag="ptc")
            conv_block(T1, 0, True, False)
            T1s = work.tile([P, cols], fp16, tag="T1s")
            nc.scalar.activation(T1s[:], T1[:], Copy)
            conv_block(pt_c, 3, True, False)
            for rsub, nr in subs:
                nc.tensor.matmul(
                    R3[:, rsub * W:(rsub + nr) * W], ident_s[:],
                    T1s[:, rsub * W:(rsub + nr) * W],
                    start=True, stop=False)
            conv_block(R3, 2, False, True, skip=True)
            conv_block(T1, 1, False, True, skip=True)
            conv_block(pt_c, 4, False, True)

            # sigmoids with fused bias; ScalarE may cross partition bases.
            # R3 (t3n) retires before T1 (t2): queue its readers first.
            I = work.tile([P, cols], fp16, tag="I")     # [i_r; i_i]
            O = work.tile([P, cols], fp16, tag="O")     # [o_r; o_i]
            O2 = work.tile([P, cols], fp16, tag="O2")   # [o_i; o_r]
            nc.scalar.activation(I[0:HALF, :], R3[0:HALF, :], Sigmoid,
                                 bias=bias_s[0:HALF, 0:1])
            nc.scalar.activation(O[0:HALF, :], R3[HALF:P, :], Sigmoid,
                                 bias=bias_s[HALF:P, 4:5])
            nc.scalar.activation(O2[HALF:P, :], R3[HALF:P, :], Sigmoid,
                                 bias=bias_s[HALF:P, 4:5])
            nc.scalar.activation(I[HALF:P, :], T1[0:HALF, :], Sigmoid,
                                 bias=bias_s[0:HALF, 3:4])
            nc.scalar.activation(O[HALF:P, :], T1[HALF:P, :], Sigmoid,
                                 bias=bias_s[HALF:P, 1:2])
            nc.scalar.activation(O2[0:HALF, :], T1[HALF:P, :], Sigmoid,
                                 bias=bias_s[HALF:P, 1:2])

            # CTs = [cti; -ctr] straight from PSUM via partition-crossed
            # activations, so no SBUF->SBUF swap DMAs are needed
            CT = work.tile([P, cols], fp16, tag="CT")      # [ctr; cti]
            nc.scalar.activation(CT[:], pt_c[:], Tanh, bias=bias_s[:, 2:3])
            CTs = work.tile([P, cols], fp16, tag="CTs")    # [cti; -ctr]
            nc.scalar.activation(CTs[0:HALF, :], pt_c[HALF:P, :], Tanh,
                                 bias=bias_s[HALF:P, 2:3])
            nc.scalar.activation(CTs[HALF:P, :], pt_c[0:HALF, :], Tanh,
                                 bias=bias_s[0:HALF, 5:6], scale=-1.0)

            # i (*) ct (complex): product halves written to base-0/base-64 so
            # every TensorTensor keeps same-base inputs
            P1 = work.tile([P, cols], fp16, tag="P1")      # [ir*ctr ; ir*cti]
            nc.vector.tensor_mul(P1[0:HALF, :], I[0:HALF, :], CT[0:HALF, :])
            nc.vector.tensor_mul(P1[HALF:P, :], I[0:HALF, :], CTs[0:HALF, :])
            P2 = work.tile([P, cols], fp16, tag="P2")      # [ii*cti ; -ii*ctr]
            nc.vector.tensor_mul(P2[0:HALF, :], I[HALF:P, :], CT[HALF:P, :])
            nc.vector.tensor_mul(P2[HALF:P, :], I[HALF:P, :], CTs[HALF:P, :])
            tmp = work.tile([P, cols], fp16, tag="tmp")
            nc.vector.tensor_sub(tmp[:], P1[:], P2[:])

            xc_t = work.tile([P, cols], fp16, tag="xc_t")
            nc.sync.dma_start(xc_t[:], xc_d[:, b, r0:r0 + nrows, :])
            cnew = work.tile([P, cols], fp16, tag="cnew")
            nc.vector.tensor_add(cnew[:], xc_t[:], tmp[:])
            nc.sync.dma_start(c_d[:, b, r0:r0 + nrows, :], cnew[:])

            T = work.tile([P, cols], fp16, tag="T")        # [tr; ti]
            nc.scalar.activation(T[:], cnew[:], Tanh)
            Tn = work.tile([P, cols], fp16, tag="Tn")      # [.. ; -ti]
            nc.scalar.activation(Tn[HALF:P, :], cnew[HALF:P, :], Tanh,
                                 scale=-1.0)

            Q1 = work.tile([P, cols], fp16, tag="Q1")      # [or*tr ; oi*tr]
            nc.vector.tensor_mul(Q1[0:HALF, :], O[0:HALF, :], T[0:HALF, :])
            nc.vector.tensor_mul(Q1[HALF:P, :], O2[0:HALF, :], T[0:HALF, :])
            Q2 = work.tile([P, cols], fp16, tag="Q2")      # [oi*ti ; -or*ti]
            nc.vector.tensor_mul(Q2[0:HALF, :], O[HALF:P, :], T[HALF:P, :])
            nc.vector.tensor_mul(Q2[HALF:P, :], O2[HALF:P, :], Tn[HALF:P, :])

            hnew = work.tile([P, cols], fp16, tag="hnew")
            nc.vector.tensor_sub(hnew[:], Q1[:], Q2[:])
            nc.sync.dma_start(h_d[:, b, r0:r0 + nrows, :], hnew[:])
            return last_mm[0]

        # uniform 512-col chunks (1 PSUM bank per accumulator, so all three
        # pools double-buffer in 6 of 8 banks); tiny last chunk shortens the
        # post-matmul epilogue chain; z-chunk loads run two tiles ahead
        SCHEDULE = ([(0, r, 8) for r in range(0, 64, 8)]
                    + [(1, r, 8) for r in range(0, 56, 8)]
                    + [(1, 56, 4), (1, 60, 4)])
        deferred = {1: [(0, 'B')], 2: [(0, 'C')], 5: [(1, 'A')],
                    7: [(1, 'B')], 9: [(1, 'C')]}
        anchor = None
        for tidx, (b, r0, nrows) in enumerate(SCHEDULE):
            for b2, ch2 in deferred.pop(tidx, []):
                load_z(b2, ch2, after=anchor)
            anchor = macro_tile(b, r0, nrows)

    _split_excess_waits(nc)
    return nc


def _prep_inputs(inputs):
    """Host-side shard + layout prep. Returns per-core in_maps."""
    f16 = np.float16
    x = np.asarray(inputs['x'], np.float32)
    h_prev = np.asarray(inputs['h_prev'], np.float32)
    c_prev = np.asarray(inputs['c_prev'], np.float32)

    xr, xi = x[:, :HALF], x[:, HALF:]
    hr, hi = h_prev[:, :HALF], h_prev[:, HALF:]
    cr, ci = c_prev[:, :HALF], c_prev[:, HALF:]

    # combined conv input, channel-major, zero-padded, fp16: [128, B, 66, 66]
    def prep_z(a):
        z = a.transpose(1, 0, 2, 3)
        return np.pad(z, ((0, 0), (0, 0), (1, 1), (1, 1))).astype(f16)
    zr_f = np.concatenate([xr, hr], axis=1)
    zi_f = np.concatenate([xi, hi], axis=1)
    zr = prep_z(zr_f)
    zi = prep_z(zi_f)
    zs = prep_z(zr_f + zi_f)

    # x (*) c_prev (complex elementwise), channel-major fp16: [128, B, 64, 64]
    xc = np.concatenate([xr * cr - xi * ci, xr * ci + xi * cr],
                        axis=1).transpose(1, 0, 2, 3).astype(f16)

    # packed gate weights: [cin 128, 45, cout 128] fp16.
    # blocks of 9 taps: io-gauss t1 (Wr), t2 (Wi-Wr), t3 (Wr+Wi) with halves
    # [i | o]; then c-direct [Wr_c | Wi_c] on zr and [-Wi_c | Wr_c] on zi.
    Wg = {}
    for gn in ('i', 'o', 'c'):
        Wg['r' + gn] = np.asarray(inputs['Wr_' + gn], np.float32)  # [64,128,3,3]
        Wg['i' + gn] = np.asarray(inputs['Wi_' + gn], np.float32)
    wts = np.empty((NM, P, P), np.float32)
    for t in range(9):
        kh, kw = t // 3, t % 3
        for blk, (li, lo) in enumerate((
                (Wg['ri'], Wg['ro']),                          # t1: Wr
                (Wg['ii'] - Wg['ri'], Wg['io'] - Wg['ro']),    # t2: Wi-Wr
                (-Wg['ri'] - Wg['ii'], -Wg['ro'] - Wg['io']),  # -t3: -(Wr+Wi)
                (Wg['rc'], Wg['ic']),                          # c on zr
                (-Wg['ic'], Wg['rc']))):                       # c on zi
            wts[blk * 9 + t, :, :HALF] = li[:, :, kh, kw].T
            wts[blk * 9 + t, :, HALF:] = lo[:, :, kh, kw].T
    wts = np.ascontiguousarray(wts.transpose(1, 0, 2)).astype(f16)

    # bias columns: 0:[br_i;bi_i] 1:[br_o;bi_o] 2:[br_c;bi_c]
    #               3:[bi_i;br_i] 4:[bi_o;br_o] 5:-col2
    bias = np.empty((P, 6), np.float32)
    for g, gn in enumerate('ioc'):
        br = np.asarray(inputs['br_' + gn], np.float32)
        bi = np.asarray(inputs['bi_' + gn], np.float32)
        bias[:, g] = np.concatenate([br, bi])
        if gn != 'c':
            bias[:, 3 + g] = np.concatenate([bi, br])
    bias[:, 5] = -bias[:, 2]

    in_maps = []
    for c in range(N_CORES):
        sl = slice(c * B_CORE, (c + 1) * B_CORE)
        in_maps.append({
            "zs": np.ascontiguousarray(zs[:, sl]),
            "zr": np.ascontiguousarray(zr[:, sl]),
            "zi": np.ascontiguousarray(zi[:, sl]),
            "wts": wts,
            "bias": bias,
            "xc": np.ascontiguousarray(xc[:, sl]),
            "ident": np.eye(P, dtype=f16),
        })
    return in_maps


def _gather_outputs(results):
    h_full = np.empty((B, P, H, W), np.float32)
    c_full = np.empty((B, P, H, W), np.float32)
    for c in range(N_CORES):
        sl = slice(c * B_CORE, (c + 1) * B_CORE)
        h_full[sl] = results[c]["h_out"].transpose(1, 0, 2, 3).astype(np.float32)
        c_full[sl] = results[c]["c_out"].transpose(1, 0, 2, 3).astype(np.float32)
    return h_full, c_full


def _run(inputs, trace=False, trace_kwargs=None):
    from concourse.bass_utils import run_bass_kernel_spmd

    if "nc" not in _CACHE:
        _CACHE["nc"] = _build_program()
    nc = _CACHE["nc"]
    in_maps = _prep_inputs(inputs)
    r = run_bass_kernel_spmd(nc, in_maps, list(range(N_CORES)),
                             trace=trace, trace_kwargs=trace_kwargs or {})
    return _gather_outputs(r.results), r


def kernel(**inputs):
    (h_full, c_full), _ = _run(inputs)
    return h_full, c_full


# revision 25
# speedup vs baseline: 1.1806x; 1.0065x over previous
"""ConvLSTM cell (complex-valued gates) on 8 TRN2 NeuronCores.

Strategy
--------
Data-parallel over batch: 16 images -> 2 per core. Per core, the three
live gates' complex 3x3 convs are computed as shifted matmuls
accumulated in PSUM:

    out[128, 512] += lhsT[128in, 128out].T @ z_shift[128in, 512]

The i and o gates (both sigmoid) are computed with the Gauss 3-mult
complex trick, packed pairwise so every pass keeps the full 128-wide
PE output:

    t1 = conv(zr+zi, Wr)        t2 = conv(zr, Wi-Wr)   t3 = conv(zi, Wr+Wi)
    y_r = t1 - t3               y_i = t1 + t2

with lhsT halves [t_i | t_o]. That is 3 passes per tap for both gates
vs 4 direct. The c gate stays direct (2 passes/tap, [re|im] packed).
Total 45 PE passes/tap-set vs 54 direct: ~154 us tensor-engine floor.

The Gauss combines are folded into PSUM accumulation so the epilogue
stays as cheap as the direct version (extra DVE/ScalarE traffic was
measured to back-pressure the PE stream): t1 accumulates in T1, one
ScalarE copy duplicates it to R3, then t2 accumulates on top of T1
(-> y_i) and -t3 (host-negated weights) on top of R3 (-> y_r), with
the c-gate's zr passes filling the PE while the copy drains.

All matmul operands are fp16 (full PE speed). ScalarE applies
sigmoid/tanh from PSUM/SBUF with the per-channel bias fused. VectorE
does the Gauss combines (PSUM->SBUF) and the complex elementwise update
in fp16. x (*) c_prev is an input-only elementwise term precomputed on
the host and added on-chip. Outputs leave as fp16, upcast on host.

The spatial dim is processed in 10 macro-tiles per core (8..16 rows x
1024 cols max), 4 PSUM accumulation tiles (t1/t2/t3/c) per chunk.
z is kept resident in SBUF, zero-padded to 66x66 on the host so conv
taps are plain shifted access patterns. Weight DMA is split into
per-stream blocks in first-use order so the first matmul starts early.
"""
import sys
import numpy as np

sys.path.insert(0, "/opt/trn_rl_repo")

P = 128          # partitions / channels (64 real + 64 imag)
HALF = 64
B = 16           # full batch
N_CORES = 8
B_CORE = B // N_CORES   # batch per core
H = W = 64
HP = WP = 66     # padded spatial
MACRO = 16 * W   # max columns per macro tile
NM = 45          # packed weight passes: 9 taps x (3 io-gauss + 2 c-direct)

_CACHE = {}


def _apply_drain_patch(tile_mod):
    """The kernel-tail drain aggregates one wait per live proc-semaphore, but
    walrus rejects instructions with more than a few sync waits. Split the
    tail waits across a chain of single-wait drains."""
    if getattr(tile_mod.TileContext, "_drain_patched", False):
        return

    def _patched(self, tick_clock, wait_clock):
        ScopedClock = tile_mod.ScopedClock
        nc = self.nc
        drain_inst = nc.sync.drain()
        wait_clock.add_sem_waits(
            drain_inst.ins, ScopedClock({None: tick_clock.global_clock})
        )
        NW = 3
        si = drain_inst.ins.sync_info
        if si is not None and si.on_wait and len(si.on_wait) > NW:
            conds = list(si.on_wait)
            si.on_wait = conds[:NW]
            rest = conds[NW:]
            while rest:
                extra = nc.sync.drain()
                esi = extra.ins.sync_info
                if esi is None:
                    import bass_rust
                    extra.ins.sync_info = bass_rust.SyncInfo(
                        on_wait=rest[:NW], on_update=[])
                else:
                    esi.on_wait = rest[:NW]
                rest = rest[NW:]

        nc.all_engine_barrier()
        assert self.sems is not None
        popped = nc._tile_sem_poison_stack.pop()
        assert popped is self._sem_poison
        nc.clear_and_free_semaphores(list(self.sems.allocated().values()))
        nc.all_engine_barrier()

    tile_mod.TileContext._drain_and_barrier = _patched
    tile_mod.TileContext._drain_patched = True


def _split_excess_waits(nc, max_waits=1):
    """walrus's per-instruction sync-wait slots are tight (1 for some ISA
    structs). Hoist excess waits into same-engine no-ops inserted directly
    before the instruction — identical semantics, per-engine order kept."""
    import concourse.mybir as mybir
    n_new = 0
    for fn in nc.m.functions:
        for bb in fn.blocks:
            il = bb.instructions
            out = []
            for inst in il:
                si = inst.sync_info
                if si is not None and si.on_wait and len(si.on_wait) > max_waits:
                    conds = list(si.on_wait)
                    si.on_wait = conds[:max_waits]
                    rest = conds[max_waits:]
                    for j in range(0, len(rest), max_waits):
                        nop = mybir.InstNoOp(
                            name=f"{inst.name}_w{j}",
                            sync_info=mybir.SyncInfo(
                                on_wait=rest[j:j + max_waits], on_update=[]),
                            bass_nofuse=True,
                            engine=inst.engine,
                        )
                        out.append(nop)
                        n_new += 1
                out.append(inst)
            if n_new:
                il[:] = out
    return n_new


def _build_program():
    import concourse.bass as bass
    import concourse.tile as tile
    from concourse import mybir
    from contextlib import ExitStack

    _apply_drain_patch(tile)
    fp16 = mybir.dt.float16
    f32 = mybir.dt.float32
    Sigmoid = mybir.ActivationFunctionType.Sigmoid
    Tanh = mybir.ActivationFunctionType.Tanh
    Copy = mybir.ActivationFunctionType.Copy

    nc = bass.Bass("TRN2", target_bir_lowering=False, debug=False)
    zs_d = nc.dram_tensor("zs", [P, B_CORE, HP, WP], fp16, kind="ExternalInput").ap()
    zr_d = nc.dram_tensor("zr", [P, B_CORE, HP, WP], fp16, kind="ExternalInput").ap()
    zi_d = nc.dram_tensor("zi", [P, B_CORE, HP, WP], fp16, kind="ExternalInput").ap()
    w_d = nc.dram_tensor("wts", [P, NM, P], fp16, kind="ExternalInput").ap()
    b_d = nc.dram_tensor("bias", [P, 6], f32, kind="ExternalInput").ap()
    xc_d = nc.dram_tensor("xc", [P, B_CORE, H, W], fp16, kind="ExternalInput").ap()
    id_d = nc.dram_tensor("ident", [P, P], fp16, kind="ExternalInput").ap()
    h_d = nc.dram_tensor("h_out", [P, B_CORE, H, W], fp16, kind="ExternalOutput").ap()
    c_d = nc.dram_tensor("c_out", [P, B_CORE, H, W], fp16, kind="ExternalOutput").ap()

    # padded-row chunks (overlapping): A=[0:18) B=[16:34) C=[32:66)
    Z_CHUNKS = {'A': (0, 18), 'B': (16, 18), 'C': (32, 34)}
    # matmul streams: (z component, weight-block offset). io-gauss t1/t2/t3
    # then c-direct zr/zi; per block 9 taps.
    STREAMS = [('s', 0), ('r', 9), ('i', 18), ('r', 27), ('i', 36)]

    with tile.TileContext(nc) as tc, ExitStack() as ctx:
        const = ctx.enter_context(tc.tile_pool(name="const", bufs=1))
        w_s = const.tile([P, NM, P], fp16, name="wts")
        z_ch = {}

        def load_w(blk, after=None):
            dm = nc.sync.dma_start(w_s[:, blk * 9:(blk + 1) * 9, :],
                                   w_d[:, blk * 9:(blk + 1) * 9, :])
            if after is not None:
                tile.add_dep_helper(dm.ins, after, reason="defer w block")
            return dm

        def load_z(b, ch, comps='sri', after=None):
            row0, nr = Z_CHUNKS[ch]
            for comp in comps:
                zt_d = {'s': zs_d, 'r': zr_d, 'i': zi_d}[comp]
                t = const.tile([P, nr, WP], fp16, name=f"z{comp}_{b}_{ch}")
                dm = nc.sync.dma_start(t[:], zt_d[:, b, row0:row0 + nr, :])
                if after is not None:
                    # hold the transfer back until the anchor matmul retires so
                    # it can't steal HBM bandwidth from earlier-needed loads
                    tile.add_dep_helper(dm.ins, after,
                                        reason="defer non-critical z load")
                z_ch[(comp, b, ch)] = t

        # upfront: ONLY what the first two conv blocks need (~1.2MB); the
        # rest is anchored on chunk-0 matmuls so the DGE round-robin can't
        # slow the critical transfers down
        load_w(0)
        load_z(0, 'A', comps='s')
        load_z(0, 'A', comps='r')
        load_w(3)
        ident_s = const.tile([P, P], fp16, name="ident")
        bias_s = const.tile([P, 6], f32)

        def load_rest_1(anchor):
            load_z(0, 'A', comps='i', after=anchor)
            load_w(2, after=anchor)
            dm = nc.sync.dma_start(ident_s[:], id_d[:])
            tile.add_dep_helper(dm.ins, anchor, reason="defer ident")

        def load_rest_2(anchor):
            load_w(1, after=anchor)
            load_w(4, after=anchor)
            dm = nc.sync.dma_start(bias_s[:], b_d[:])
            tile.add_dep_helper(dm.ins, anchor, reason="defer bias")

        ps_1 = ctx.enter_context(tc.tile_pool(name="ps_1", bufs=2, space="PSUM"))
        ps_3 = ctx.enter_context(tc.tile_pool(name="ps_3", bufs=2, space="PSUM"))
        ps_c = ctx.enter_context(tc.tile_pool(name="ps_c", bufs=2, space="PSUM"))
        work = ctx.enter_context(tc.tile_pool(name="work", bufs=2))

        def macro_tile(b, r0, nrows, hooks=(), io_last=False):
            cols = nrows * W
            if r0 + nrows + 1 < 18:
                ch = 'A'
            elif r0 >= 16 and r0 + nrows + 1 < 34:
                ch = 'B'
            else:
                ch = 'C'
            roff = Z_CHUNKS[ch][0]   # chunk's first padded row

            last_mm = [None]

            subs = ([(i * 8, 8) for i in range(nrows // 8)]
                    if nrows >= 8 else [(0, nrows)])

            def conv_block(pt, si, start, stop, skip=False):
                comp, mbase = STREAMS[si]
                z_s = z_ch[(comp, b, ch)]
                for t in range(9):
                    kh, kw = t // 3, t % 3
                    for rsub, nr in subs:
                        r0h = r0 + rsub - roff
                        mm = nc.tensor.matmul(
                            pt[:, rsub * W:(rsub + nr) * W],
                            w_s[:, mbase + t, :],
                            z_s[:, r0h + kh:r0h + kh + nr, kw:kw + 64],
                            start=(start and t == 0), stop=(stop and t == 8),
                            skip_group_check=skip,
                        )
                        last_mm[0] = mm.ins
                return pt

            # io gauss unit, combines folded into PSUM accumulation:
            #   T1 = t1 (+ t2 later) -> y_i
            #   R3 = ident @ fp16(t1), then + -t3 -> y_r
            # Both PSUM groups are conventional (one start, one stop); the
            # t1 staging copy runs on the idle GpSimd engine and the c-gate
            # zr passes fill the PE while it drains.
            T1 = ps_1.tile([P, cols], f32, tag="pt1")   # -> [y_i_i | y_i_o]
            R3 = ps_3.tile([P, cols], f32, tag="pt3")   # -> [y_r_i | y_r_o]
            pt_c = ps_c.tile([P, cols], f32, tag="ptc")
            hooks = dict(hooks)

            conv_block(T1, 0, True, False)
            if 't1_done' in hooks:
                hooks['t1_done'](last_mm[0])
            T1s = work.tile([P, cols], fp16, tag="T1s")
            nc.scalar.activation(T1s[:], T1[:], Copy)
            conv_block(pt_c, 3, True, False)
            if 'czr_done' in hooks:
                hooks['czr_done'](last_mm[0])
            if io_last:
                conv_block(pt_c, 4, False, True)

            def ident_block():
                for rsub, nr in subs:
                    nc.tensor.matmul(
                        R3[:, rsub * W:(rsub + nr) * W], ident_s[:],
                        T1s[:, rsub * W:(rsub + nr) * W],
                        start=True, stop=False)

            ident_block()
            conv_block(R3, 2, False, True, skip=True)
            conv_block(T1, 1, False, True, skip=True)
            if not io_last:
                conv_block(pt_c, 4, False, True)

            # CTs = [cti; -ctr] straight from PSUM via partition-crossed
            # activations, so no SBUF->SBUF swap DMAs are needed.  For the
            # io-last tail chunk the c gate retires first, so its tanh acts
            # go at the head of the scalar queue.
            CT = work.tile([P, cols], fp16, tag="CT")      # [ctr; cti]
            CTs = work.tile([P, cols], fp16, tag="CTs")    # [cti; -ctr]

            def c_acts():
                nc.scalar.activation(CT[:], pt_c[:], Tanh, bias=bias_s[:, 2:3])
                nc.scalar.activation(CTs[0:HALF, :], pt_c[HALF:P, :], Tanh,
                                     bias=bias_s[HALF:P, 2:3])
                nc.scalar.activation(CTs[HALF:P, :], pt_c[0:HALF, :], Tanh,
                                     bias=bias_s[0:HALF, 5:6], scale=-1.0)

            if io_last:
                c_acts()

            # sigmoids with fused bias; ScalarE may cross partition bases.
            # R3 (t3n) retires before T1 (t2): queue its readers first.
            I = work.tile([P, cols], fp16, tag="I")     # [i_r; i_i]
            O = work.tile([P, cols], fp16, tag="O")     # [o_r; o_i]
            O2 = work.tile([P, cols], fp16, tag="O2")   # [o_i; o_r]
            nc.scalar.activation(I[0:HALF, :], R3[0:HALF, :], Sigmoid,
                                 bias=bias_s[0:HALF, 0:1])
            nc.scalar.activation(O[0:HALF, :], R3[HALF:P, :], Sigmoid,
                                 bias=bias_s[HALF:P, 4:5])
            nc.scalar.activation(O2[HALF:P, :], R3[HALF:P, :], Sigmoid,
                                 bias=bias_s[HALF:P, 4:5])
            nc.scalar.activation(I[HALF:P, :], T1[0:HALF, :], Sigmoid,
                                 bias=bias_s[0:HALF, 3:4])
            nc.scalar.activation(O[HALF:P, :], T1[HALF:P, :], Sigmoid,
                                 bias=bias_s[HALF:P, 1:2])
            nc.scalar.activation(O2[0:HALF, :], T1[HALF:P, :], Sigmoid,
                                 bias=bias_s[HALF:P, 1:2])

            if not io_last:
                c_acts()

            # i (*) ct (complex): product halves written to base-0/base-64 so
            # every TensorTensor keeps same-base inputs
            P1 = work.tile([P, cols], fp16, tag="P1")      # [ir*ctr ; ir*cti]
            nc.vector.tensor_mul(P1[0:HALF, :], I[0:HALF, :], CT[0:HALF, :])
            nc.vector.tensor_mul(P1[HALF:P, :], I[0:HALF, :], CTs[0:HALF, :])
            P2 = work.tile([P, cols], fp16, tag="P2")      # [ii*cti ; -ii*ctr]
            nc.vector.tensor_mul(P2[0:HALF, :], I[HALF:P, :], CT[HALF:P, :])
            nc.vector.tensor_mul(P2[HALF:P, :], I[HALF:P, :], CTs[HALF:P, :])
            tmp = work.tile([P, cols], fp16, tag="tmp")
            nc.vector.tensor_sub(tmp[:], P1[:], P2[:])

            xc_t = work.tile([P, cols], fp16, tag="xc_t")
            nc.sync.dma_start(xc_t[:], xc_d[:, b, r0:r0 + nrows, :])
            cnew = work.tile([P, cols], fp16, tag="cnew")
            nc.vector.tensor_add(cnew[:], xc_t[:], tmp[:])
            nc.sync.dma_start(c_d[:, b, r0:r0 + nrows, :], cnew[:])

            T = work.tile([P, cols], fp16, tag="T")        # [tr; ti]
            nc.scalar.activation(T[:], cnew[:], Tanh)
            Tn = work.tile([P, cols], fp16, tag="Tn")      # [.. ; -ti]
            nc.scalar.activation(Tn[HALF:P, :], cnew[HALF:P, :], Tanh,
                                 scale=-1.0)

            Q1 = work.tile([P, cols], fp16, tag="Q1")      # [or*tr ; oi*tr]
            nc.vector.tensor_mul(Q1[0:HALF, :], O[0:HALF, :], T[0:HALF, :])
            nc.vector.tensor_mul(Q1[HALF:P, :], O2[0:HALF, :], T[0:HALF, :])
            Q2 = work.tile([P, cols], fp16, tag="Q2")      # [oi*ti ; -or*ti]
            nc.vector.tensor_mul(Q2[0:HALF, :], O[HALF:P, :], T[HALF:P, :])
            nc.vector.tensor_mul(Q2[HALF:P, :], O2[HALF:P, :], Tn[HALF:P, :])

            hnew = work.tile([P, cols], fp16, tag="hnew")
            nc.vector.tensor_sub(hnew[:], Q1[:], Q2[:])
            nc.sync.dma_start(h_d[:, b, r0:r0 + nrows, :], hnew[:])
            return last_mm[0]

        # uniform 512-col chunks (1 PSUM bank per accumulator, so all three
        # pools double-buffer in 6 of 8 banks); tiny last chunk shortens the
        # post-matmul epilogue chain; z-chunk loads run two tiles ahead
        SCHEDULE = ([(0, r, 8) for r in range(0, 64, 8)]
                    + [(1, r, 8) for r in range(0, 56, 8)]
                    + [(1, 56, 4), (1, 60, 4)])
        deferred = {1: [(0, 'B')], 2: [(0, 'C')], 5: [(1, 'A')],
                    7: [(1, 'B')], 9: [(1, 'C')]}
        anchor = None
        last_t = len(SCHEDULE) - 1
        for tidx, (b, r0, nrows) in enumerate(SCHEDULE):
            for b2, ch2 in deferred.pop(tidx, []):
                load_z(b2, ch2, after=anchor)
            hooks = ({'t1_done': load_rest_1, 'czr_done': load_rest_2}
                     if tidx == 0 else ())
            anchor = macro_tile(b, r0, nrows, hooks=hooks,
                                io_last=(tidx == last_t))

    _split_excess_waits(nc)
    return nc


def _prep_inputs(inputs):
    """Host-side shard + layout prep. Returns per-core in_maps."""
    f16 = np.float16
    x = np.asarray(inputs['x'], np.float32)
    h_prev = np.asarray(inputs['h_prev'], np.float32)
    c_prev = np.asarray(inputs['c_prev'], np.float32)

    xr, xi = x[:, :HALF], x[:, HALF:]
    hr, hi = h_prev[:, :HALF], h_prev[:, HALF:]
    cr, ci = c_prev[:, :HALF], c_prev[:, HALF:]

    # combined conv input, channel-major, zero-padded, fp16: [128, B, 66, 66]
    def prep_z(a):
        z = a.transpose(1, 0, 2, 3)
        return np.pad(z, ((0, 0), (0, 0), (1, 1), (1, 1))).astype(f16)
    zr_f = np.concatenate([xr, hr], axis=1)
    zi_f = np.concatenate([xi, hi], axis=1)
    zr = prep_z(zr_f)
    zi = prep_z(zi_f)
    zs = prep_z(zr_f + zi_f)

    # x (*) c_prev (complex elementwise), channel-major fp16: [128, B, 64, 64]
    xc = np.concatenate([xr * cr - xi * ci, xr * ci + xi * cr],
                        axis=1).transpose(1, 0, 2, 3).astype(f16)

    # packed gate weights: [cin 128, 45, cout 128] fp16.
    # blocks of 9 taps: io-gauss t1 (Wr), t2 (Wi-Wr), t3 (Wr+Wi) with halves
    # [i | o]; then c-direct [Wr_c | Wi_c] on zr and [-Wi_c | Wr_c] on zi.
    Wg = {}
    for gn in ('i', 'o', 'c'):
        Wg['r' + gn] = np.asarray(inputs['Wr_' + gn], np.float32)  # [64,128,3,3]
        Wg['i' + gn] = np.asarray(inputs['Wi_' + gn], np.float32)
    wts = np.empty((NM, P, P), np.float32)
    for t in range(9):
        kh, kw = t // 3, t % 3
        for blk, (li, lo) in enumerate((
                (Wg['ri'], Wg['ro']),                          # t1: Wr
                (Wg['ii'] - Wg['ri'], Wg['io'] - Wg['ro']),    # t2: Wi-Wr
                (-Wg['ri'] - Wg['ii'], -Wg['ro'] - Wg['io']),  # -t3: -(Wr+Wi)
                (Wg['rc'], Wg['ic']),                          # c on zr
                (-Wg['ic'], Wg['rc']))):                       # c on zi
            wts[blk * 9 + t, :, :HALF] = li[:, :, kh, kw].T
            wts[blk * 9 + t, :, HALF:] = lo[:, :, kh, kw].T
    wts = np.ascontiguousarray(wts.transpose(1, 0, 2)).astype(f16)

    # bias columns: 0:[br_i;bi_i] 1:[br_o;bi_o] 2:[br_c;bi_c]
    #               3:[bi_i;br_i] 4:[bi_o;br_o] 5:-col2
    bias = np.empty((P, 6), np.float32)
    for g, gn in enumerate('ioc'):
        br = np.asarray(inputs['br_' + gn], np.float32)
        bi = np.asarray(inputs['bi_' + gn], np.float32)
        bias[:, g] = np.concatenate([br, bi])
        if gn != 'c':
            bias[:, 3 + g] = np.concatenate([bi, br])
    bias[:, 5] = -bias[:, 2]

    in_maps = []
    for c in range(N_CORES):
        sl = slice(c * B_CORE, (c + 1) * B_CORE)
        in_maps.append({
            "zs": np.ascontiguousarray(zs[:, sl]),
            "zr": np.ascontiguousarray(zr[:, sl]),
            "zi": np.ascontiguousarray(zi[:, sl]),
            "wts": wts,
            "bias": bias,
            "xc": np.ascontiguousarray(xc[:, sl]),
            "ident": np.eye(P, dtype=f16),
        })
    return in_maps


def _gather_outputs(results):
    h_full = np.empty((B, P, H, W), np.float32)
    c_full = np.empty((B, P, H, W), np.float32)
    for c in range(N_CORES):
        sl = slice(c * B_CORE, (c + 1) * B_CORE)
        h_full[sl] = results[c]["h_out"].transpose(1, 0, 2, 3).astype(np.float32)
        c_full[sl] = results[c]["c_out"].transpose(1, 0, 2, 3).astype(np.float32)
    return h_full, c_full


def _run(inputs, trace=False, trace_kwargs=None):
    from concourse.bass_utils import run_bass_kernel_spmd

    if "nc" not in _CACHE:
        _CACHE["nc"] = _build_program()
    nc = _CACHE["nc"]
    in_maps = _prep_inputs(inputs)
    r = run_bass_kernel_spmd(nc, in_maps, list(range(N_CORES)),
                             trace=trace, trace_kwargs=trace_kwargs or {})
    return _gather_outputs(r.results), r


def kernel(**inputs):
    (h_full, c_full), _ = _run(inputs)
    return h_full, c_full


# revision 29
# speedup vs baseline: 1.1815x; 1.0007x over previous
"""ConvLSTM cell (complex-valued gates) on 8 TRN2 NeuronCores.

Strategy
--------
Data-parallel over batch: 16 images -> 2 per core. Per core, the three
live gates' complex 3x3 convs are computed as shifted matmuls
accumulated in PSUM:

    out[128, 512] += lhsT[128in, 128out].T @ z_shift[128in, 512]

The i and o gates (both sigmoid) are computed with the Gauss 3-mult
complex trick, packed pairwise so every pass keeps the full 128-wide
PE output:

    t1 = conv(zr+zi, Wr)        t2 = conv(zr, Wi-Wr)   t3 = conv(zi, Wr+Wi)
    y_r = t1 - t3               y_i = t1 + t2

with lhsT halves [t_i | t_o]. That is 3 passes per tap for both gates
vs 4 direct. The c gate stays direct (2 passes/tap, [re|im] packed).
Total 45 PE passes/tap-set vs 54 direct: ~154 us tensor-engine floor.

The Gauss combines are folded into PSUM accumulation so the epilogue
stays as cheap as the direct version (extra DVE/ScalarE traffic was
measured to back-pressure the PE stream): t1 accumulates in T1, one
ScalarE copy duplicates it to R3, then t2 accumulates on top of T1
(-> y_i) and -t3 (host-negated weights) on top of R3 (-> y_r), with
the c-gate's zr passes filling the PE while the copy drains.

All matmul operands are fp16 (full PE speed). ScalarE applies
sigmoid/tanh from PSUM/SBUF with the per-channel bias fused. VectorE
does the Gauss combines (PSUM->SBUF) and the complex elementwise update
in fp16. x (*) c_prev is an input-only elementwise term precomputed on
the host and added on-chip. Outputs leave as fp16, upcast on host.

The spatial dim is processed in 10 macro-tiles per core (8..16 rows x
1024 cols max), 4 PSUM accumulation tiles (t1/t2/t3/c) per chunk.
z is kept resident in SBUF, zero-padded to 66x66 on the host so conv
taps are plain shifted access patterns. Weight DMA is split into
per-stream blocks in first-use order so the first matmul starts early.
"""
import sys
import numpy as np

sys.path.insert(0, "/opt/trn_rl_repo")

P = 128          # partitions / channels (64 real + 64 imag)
HALF = 64
B = 16           # full batch
N_CORES = 8
B_CORE = B // N_CORES   # batch per core
H = W = 64
HP = WP = 66     # padded spatial
MACRO = 16 * W   # max columns per macro tile
NM = 45          # packed weight passes: 9 taps x (3 io-gauss + 2 c-direct)

_CACHE = {}


def _apply_drain_patch(tile_mod):
    """The kernel-tail drain aggregates one wait per live proc-semaphore, but
    walrus rejects instructions with more than a few sync waits. Split the
    tail waits across a chain of single-wait drains."""
    if getattr(tile_mod.TileContext, "_drain_patched", False):
        return

    def _patched(self, tick_clock, wait_clock):
        ScopedClock = tile_mod.ScopedClock
        nc = self.nc
        drain_inst = nc.sync.drain()
        wait_clock.add_sem_waits(
            drain_inst.ins, ScopedClock({None: tick_clock.global_clock})
        )
        NW = 3
        si = drain_inst.ins.sync_info
        if si is not None and si.on_wait and len(si.on_wait) > NW:
            conds = list(si.on_wait)
            si.on_wait = conds[:NW]
            rest = conds[NW:]
            while rest:
                extra = nc.sync.drain()
                esi = extra.ins.sync_info
                if esi is None:
                    import bass_rust
                    extra.ins.sync_info = bass_rust.SyncInfo(
                        on_wait=rest[:NW], on_update=[])
                else:
                    esi.on_wait = rest[:NW]
                rest = rest[NW:]

        nc.all_engine_barrier()
        assert self.sems is not None
        popped = nc._tile_sem_poison_stack.pop()
        assert popped is self._sem_poison
        nc.clear_and_free_semaphores(list(self.sems.allocated().values()))
        nc.all_engine_barrier()

    tile_mod.TileContext._drain_and_barrier = _patched
    tile_mod.TileContext._drain_patched = True


def _split_excess_waits(nc, max_waits=1):
    """walrus's per-instruction sync-wait slots are tight (1 for some ISA
    structs). Hoist excess waits into same-engine no-ops inserted directly
    before the instruction — identical semantics, per-engine order kept."""
    import concourse.mybir as mybir
    n_new = 0
    for fn in nc.m.functions:
        for bb in fn.blocks:
            il = bb.instructions
            out = []
            for inst in il:
                si = inst.sync_info
                if si is not None and si.on_wait and len(si.on_wait) > max_waits:
                    conds = list(si.on_wait)
                    si.on_wait = conds[:max_waits]
                    rest = conds[max_waits:]
                    for j in range(0, len(rest), max_waits):
                        nop = mybir.InstNoOp(
                            name=f"{inst.name}_w{j}",
                            sync_info=mybir.SyncInfo(
                                on_wait=rest[j:j + max_waits], on_update=[]),
                            bass_nofuse=True,
                            engine=inst.engine,
                        )
                        out.append(nop)
                        n_new += 1
                out.append(inst)
            if n_new:
                il[:] = out
    return n_new


def _build_program():
    import concourse.bass as bass
    import concourse.tile as tile
    from concourse import mybir
    from contextlib import ExitStack

    _apply_drain_patch(tile)
    fp16 = mybir.dt.float16
    f32 = mybir.dt.float32
    Sigmoid = mybir.ActivationFunctionType.Sigmoid
    Tanh = mybir.ActivationFunctionType.Tanh
    Copy = mybir.ActivationFunctionType.Copy

    nc = bass.Bass("TRN2", target_bir_lowering=False, debug=False)
    zs_d = nc.dram_tensor("zs", [P, B_CORE, HP, WP], fp16, kind="ExternalInput").ap()
    zr_d = nc.dram_tensor("zr", [P, B_CORE, HP, WP], fp16, kind="ExternalInput").ap()
    zi_d = nc.dram_tensor("zi", [P, B_CORE, HP, WP], fp16, kind="ExternalInput").ap()
    w_d = nc.dram_tensor("wts", [P, NM, P], fp16, kind="ExternalInput").ap()
    b_d = nc.dram_tensor("bias", [P, 6], f32, kind="ExternalInput").ap()
    xc_d = nc.dram_tensor("xc", [P, B_CORE, H, W], fp16, kind="ExternalInput").ap()
    id_d = nc.dram_tensor("ident", [P, P], fp16, kind="ExternalInput").ap()
    h_d = nc.dram_tensor("h_out", [P, B_CORE, H, W], fp16, kind="ExternalOutput").ap()
    c_d = nc.dram_tensor("c_out", [P, B_CORE, H, W], fp16, kind="ExternalOutput").ap()

    # padded-row chunks (overlapping): A=[0:18) B=[16:34) C=[32:66)
    Z_CHUNKS = {'A': (0, 18), 'B': (16, 18), 'C': (32, 34)}
    # matmul streams: (z component, weight-block offset). io-gauss t1/t2/t3
    # then c-direct zr/zi; per block 9 taps.
    STREAMS = [('s', 0), ('r', 9), ('i', 18), ('r', 27), ('i', 36)]

    with tile.TileContext(nc) as tc, ExitStack() as ctx:
        const = ctx.enter_context(tc.tile_pool(name="const", bufs=1))
        w_s = const.tile([P, NM, P], fp16, name="wts")
        z_ch = {}

        def load_w(blk, after=None):
            dm = nc.sync.dma_start(w_s[:, blk * 9:(blk + 1) * 9, :],
                                   w_d[:, blk * 9:(blk + 1) * 9, :])
            if after is not None:
                tile.add_dep_helper(dm.ins, after, reason="defer w block")
            return dm

        def load_z(b, ch, comps='sri', after=None):
            row0, nr = Z_CHUNKS[ch]
            for comp in comps:
                zt_d = {'s': zs_d, 'r': zr_d, 'i': zi_d}[comp]
                t = const.tile([P, nr, WP], fp16, name=f"z{comp}_{b}_{ch}")
                dm = nc.sync.dma_start(t[:], zt_d[:, b, row0:row0 + nr, :])
                if after is not None:
                    # hold the transfer back until the anchor matmul retires so
                    # it can't steal HBM bandwidth from earlier-needed loads
                    tile.add_dep_helper(dm.ins, after,
                                        reason="defer non-critical z load")
                z_ch[(comp, b, ch)] = t

        # upfront: ONLY what the first two conv blocks need (~1.2MB); the
        # rest is anchored on chunk-0 matmuls so the DGE round-robin can't
        # slow the critical transfers down
        load_w(0)
        load_z(0, 'A', comps='s')
        load_z(0, 'A', comps='r')
        load_w(3)
        ident_s = const.tile([P, P], fp16, name="ident")
        bias_s = const.tile([P, 6], f32)

        def load_rest_1(anchor):
            load_z(0, 'A', comps='i', after=anchor)
            load_w(2, after=anchor)
            dm = nc.sync.dma_start(ident_s[:], id_d[:])
            tile.add_dep_helper(dm.ins, anchor, reason="defer ident")

        def load_rest_2(anchor):
            load_w(1, after=anchor)
            load_w(4, after=anchor)
            dm = nc.sync.dma_start(bias_s[:], b_d[:])
            tile.add_dep_helper(dm.ins, anchor, reason="defer bias")

        ps_1 = ctx.enter_context(tc.tile_pool(name="ps_1", bufs=2, space="PSUM"))
        ps_3 = ctx.enter_context(tc.tile_pool(name="ps_3", bufs=2, space="PSUM"))
        ps_c = ctx.enter_context(tc.tile_pool(name="ps_c", bufs=2, space="PSUM"))
        ps_w = ctx.enter_context(tc.tile_pool(name="ps_w", bufs=1, space="PSUM"))
        work = ctx.enter_context(tc.tile_pool(name="work", bufs=2))

        # warm the PE power-state during the startup DMA wait: the first ~8
        # matmuls otherwise run ~60% slower while the array ramps
        zero_s = const.tile([P, 512], fp16, name="zero_s")
        nc.gpsimd.memset(zero_s[:], 0.0)
        warm = ps_w.tile([P, 512], f32, name="warm")
        for _ in range(6):
            nc.tensor.matmul(warm[:], zero_s[:, 0:P], zero_s[:],
                             start=True, stop=True)

        def macro_tile(b, r0, nrows, hooks=(), io_last=False):
            cols = nrows * W
            if r0 + nrows + 1 < 18:
                ch = 'A'
            elif r0 >= 16 and r0 + nrows + 1 < 34:
                ch = 'B'
            else:
                ch = 'C'
            roff = Z_CHUNKS[ch][0]   # chunk's first padded row

            last_mm = [None]

            subs = ([(i * 8, 8) for i in range(nrows // 8)]
                    if nrows >= 8 else [(0, nrows)])

            def conv_block(pt, si, start, stop, skip=False):
                comp, mbase = STREAMS[si]
                z_s = z_ch[(comp, b, ch)]
                for t in range(9):
                    kh, kw = t // 3, t % 3
                    for rsub, nr in subs:
                        r0h = r0 + rsub - roff
                        mm = nc.tensor.matmul(
                            pt[:, rsub * W:(rsub + nr) * W],
                            w_s[:, mbase + t, :],
                            z_s[:, r0h + kh:r0h + kh + nr, kw:kw + 64],
                            start=(start and t == 0), stop=(stop and t == 8),
                            skip_group_check=skip,
                        )
                        last_mm[0] = mm.ins
                return pt

            # io gauss unit, combines folded into PSUM accumulation:
            #   T1 = t1 (+ t2 later) -> y_i
            #   R3 = ident @ fp16(t1), then + -t3 -> y_r
            # Both PSUM groups are conventional (one start, one stop); the
            # t1 staging copy runs on the idle GpSimd engine and the c-gate
            # zr passes fill the PE while it drains.
            T1 = ps_1.tile([P, cols], f32, tag="pt1")   # -> [y_i_i | y_i_o]
            R3 = ps_3.tile([P, cols], f32, tag="pt3")   # -> [y_r_i | y_r_o]
            pt_c = ps_c.tile([P, cols], f32, tag="ptc")
            hooks = dict(hooks)

            conv_block(T1, 0, True, False)
            if 't1_done' in hooks:
                hooks['t1_done'](last_mm[0])
            T1s = work.tile([P, cols], fp16, tag="T1s")
            nc.vector.tensor_copy(T1s[:], T1[:])
            conv_block(pt_c, 3, True, False)
            if 'czr_done' in hooks:
                hooks['czr_done'](last_mm[0])
            if io_last:
                conv_block(pt_c, 4, False, True)

            def ident_block():
                for rsub, nr in subs:
                    nc.tensor.matmul(
                        R3[:, rsub * W:(rsub + nr) * W], ident_s[:],
                        T1s[:, rsub * W:(rsub + nr) * W],
                        start=True, stop=False)

            ident_block()
            conv_block(R3, 2, False, True, skip=True)
            conv_block(T1, 1, False, True, skip=True)
            if not io_last:
                conv_block(pt_c, 4, False, True)

            # CTs = [cti; -ctr] straight from PSUM via partition-crossed
            # activations, so no SBUF->SBUF swap DMAs are needed.  For the
            # io-last tail chunk the c gate retires first, so its tanh acts
            # go at the head of the scalar queue.
            CT = work.tile([P, cols], fp16, tag="CT")      # [ctr; cti]
            CTs = work.tile([P, cols], fp16, tag="CTs")    # [cti; -ctr]

            def c_acts():
                nc.scalar.activation(CT[:], pt_c[:], Tanh, bias=bias_s[:, 2:3])
                nc.scalar.activation(CTs[0:HALF, :], pt_c[HALF:P, :], Tanh,
                                     bias=bias_s[HALF:P, 2:3])
                nc.scalar.activation(CTs[HALF:P, :], pt_c[0:HALF, :], Tanh,
                                     bias=bias_s[0:HALF, 5:6], scale=-1.0)

            if io_last:
                c_acts()

            # sigmoids with fused bias; ScalarE may cross partition bases.
            # R3 (t3n) retires before T1 (t2): queue its readers first.  For
            # the tail chunk, O1/O2a move behind T so the critical chain
            # I1 -> P2 -> cnew -> T is not head-blocked in the scalar queue.
            I = work.tile([P, cols], fp16, tag="I")     # [i_r; i_i]
            O = work.tile([P, cols], fp16, tag="O")     # [o_r; o_i]
            O2 = work.tile([P, cols], fp16, tag="O2")   # [o_i; o_r]
            nc.scalar.activation(I[0:HALF, :], R3[0:HALF, :], Sigmoid,
                                 bias=bias_s[0:HALF, 0:1])
            nc.scalar.activation(O[0:HALF, :], R3[HALF:P, :], Sigmoid,
                                 bias=bias_s[HALF:P, 4:5])
            nc.scalar.activation(O2[HALF:P, :], R3[HALF:P, :], Sigmoid,
                                 bias=bias_s[HALF:P, 4:5])
            nc.scalar.activation(I[HALF:P, :], T1[0:HALF, :], Sigmoid,
                                 bias=bias_s[0:HALF, 3:4])

            def late_sigmoids():
                nc.scalar.activation(O[HALF:P, :], T1[HALF:P, :], Sigmoid,
                                     bias=bias_s[HALF:P, 1:2])
                nc.scalar.activation(O2[0:HALF, :], T1[HALF:P, :], Sigmoid,
                                     bias=bias_s[HALF:P, 1:2])

            if not io_last:
                late_sigmoids()
                c_acts()

            # i (*) ct (complex): product halves written to base-0/base-64 so
            # every TensorTensor keeps same-base inputs
            P1 = work.tile([P, cols], fp16, tag="P1")      # [ir*ctr ; ir*cti]
            nc.vector.tensor_mul(P1[0:HALF, :], I[0:HALF, :], CT[0:HALF, :])
            nc.vector.tensor_mul(P1[HALF:P, :], I[0:HALF, :], CTs[0:HALF, :])
            P2 = work.tile([P, cols], fp16, tag="P2")      # [ii*cti ; -ii*ctr]
            nc.vector.tensor_mul(P2[0:HALF, :], I[HALF:P, :], CT[HALF:P, :])
            nc.vector.tensor_mul(P2[HALF:P, :], I[HALF:P, :], CTs[HALF:P, :])
            tmp = work.tile([P, cols], fp16, tag="tmp")
            nc.vector.tensor_sub(tmp[:], P1[:], P2[:])

            xc_t = work.tile([P, cols], fp16, tag="xc_t")
            nc.sync.dma_start(xc_t[:], xc_d[:, b, r0:r0 + nrows, :])
            cnew = work.tile([P, cols], fp16, tag="cnew")
            nc.vector.tensor_add(cnew[:], xc_t[:], tmp[:])
            nc.sync.dma_start(c_d[:, b, r0:r0 + nrows, :], cnew[:])

            T = work.tile([P, cols], fp16, tag="T")        # [tr; ti]
            nc.scalar.activation(T[:], cnew[:], Tanh)
            if io_last:
                late_sigmoids()

            Q1 = work.tile([P, cols], fp16, tag="Q1")      # [or*tr ; oi*tr]
            nc.vector.tensor_mul(Q1[0:HALF, :], O[0:HALF, :], T[0:HALF, :])
            nc.vector.tensor_mul(Q1[HALF:P, :], O2[0:HALF, :], T[0:HALF, :])
            Q2 = work.tile([P, cols], fp16, tag="Q2")      # [oi*ti ; or*ti]
            nc.vector.tensor_mul(Q2[0:HALF, :], O[HALF:P, :], T[HALF:P, :])
            nc.vector.tensor_mul(Q2[HALF:P, :], O2[HALF:P, :], T[HALF:P, :])

            # h = [or*tr - oi*ti ; oi*tr + or*ti]: split halves instead of a
            # negated-tanh tile, saving one ScalarE act per chunk
            hnew = work.tile([P, cols], fp16, tag="hnew")
            nc.vector.tensor_sub(hnew[0:HALF, :], Q1[0:HALF, :], Q2[0:HALF, :])
            nc.vector.tensor_add(hnew[HALF:P, :], Q1[HALF:P, :], Q2[HALF:P, :])
            nc.sync.dma_start(h_d[:, b, r0:r0 + nrows, :], hnew[:])
            return last_mm[0]

        # uniform 512-col chunks (1 PSUM bank per accumulator, so all three
        # pools double-buffer in 6 of 8 banks); tiny last chunk shortens the
        # post-matmul epilogue chain; z-chunk loads run two tiles ahead
        SCHEDULE = ([(0, r, 8) for r in range(0, 64, 8)]
                    + [(1, r, 8) for r in range(0, 56, 8)]
                    + [(1, 56, 4), (1, 60, 4)])
        deferred = {1: [(0, 'B')], 2: [(0, 'C')], 5: [(1, 'A')],
                    7: [(1, 'B')], 9: [(1, 'C')]}
        anchor = None
        last_t = len(SCHEDULE) - 1
        for tidx, (b, r0, nrows) in enumerate(SCHEDULE):
            for b2, ch2 in deferred.pop(tidx, []):
                load_z(b2, ch2, after=anchor)
            hooks = ({'t1_done': load_rest_1, 'czr_done': load_rest_2}
                     if tidx == 0 else ())
            anchor = macro_tile(b, r0, nrows, hooks=hooks,
                                io_last=(tidx == last_t))

    _split_excess_waits(nc)
    return nc


def _prep_inputs(inputs):
    """Host-side shard + layout prep. Returns per-core in_maps."""
    f16 = np.float16
    x = np.asarray(inputs['x'], np.float32)
    h_prev = np.asarray(inputs['h_prev'], np.float32)
    c_prev = np.asarray(inputs['c_prev'], np.float32)

    xr, xi = x[:, :HALF], x[:, HALF:]
    hr, hi = h_prev[:, :HALF], h_prev[:, HALF:]
    cr, ci = c_prev[:, :HALF], c_prev[:, HALF:]

    # combined conv input, channel-major, zero-padded, fp16: [128, B, 66, 66]
    def prep_z(a):
        z = a.transpose(1, 0, 2, 3)
        return np.pad(z, ((0, 0), (0, 0), (1, 1), (1, 1))).astype(f16)
    zr_f = np.concatenate([xr, hr], axis=1)
    zi_f = np.concatenate([xi, hi], axis=1)
    zr = prep_z(zr_f)
    zi = prep_z(zi_f)
    zs = prep_z(zr_f + zi_f)

    # x (*) c_prev (complex elementwise), channel-major fp16: [128, B, 64, 64]
    xc = np.concatenate([xr * cr - xi * ci, xr * ci + xi * cr],
                        axis=1).transpose(1, 0, 2, 3).astype(f16)

    # packed gate weights: [cin 128, 45, cout 128] fp16.
    # blocks of 9 taps: io-gauss t1 (Wr), t2 (Wi-Wr), t3 (Wr+Wi) with halves
    # [i | o]; then c-direct [Wr_c | Wi_c] on zr and [-Wi_c | Wr_c] on zi.
    Wg = {}
    for gn in ('i', 'o', 'c'):
        Wg['r' + gn] = np.asarray(inputs['Wr_' + gn], np.float32)  # [64,128,3,3]
        Wg['i' + gn] = np.asarray(inputs['Wi_' + gn], np.float32)
    wts = np.empty((NM, P, P), np.float32)
    for t in range(9):
        kh, kw = t // 3, t % 3
        for blk, (li, lo) in enumerate((
                (Wg['ri'], Wg['ro']),                          # t1: Wr
                (Wg['ii'] - Wg['ri'], Wg['io'] - Wg['ro']),    # t2: Wi-Wr
                (-Wg['ri'] - Wg['ii'], -Wg['ro'] - Wg['io']),  # -t3: -(Wr+Wi)
                (Wg['rc'], Wg['ic']),                          # c on zr
                (-Wg['ic'], Wg['rc']))):                       # c on zi
            wts[blk * 9 + t, :, :HALF] = li[:, :, kh, kw].T
            wts[blk * 9 + t, :, HALF:] = lo[:, :, kh, kw].T
    wts = np.ascontiguousarray(wts.transpose(1, 0, 2)).astype(f16)

    # bias columns: 0:[br_i;bi_i] 1:[br_o;bi_o] 2:[br_c;bi_c]
    #               3:[bi_i;br_i] 4:[bi_o;br_o] 5:-col2
    bias = np.empty((P, 6), np.float32)
    for g, gn in enumerate('ioc'):
        br = np.asarray(inputs['br_' + gn], np.float32)
        bi = np.asarray(inputs['bi_' + gn], np.float32)
        bias[:, g] = np.concatenate([br, bi])
        if gn != 'c':
            bias[:, 3 + g] = np.concatenate([bi, br])
    bias[:, 5] = -bias[:, 2]

    in_maps = []
    for c in range(N_CORES):
        sl = slice(c * B_CORE, (c + 1) * B_CORE)
        in_maps.append({
            "zs": np.ascontiguousarray(zs[:, sl]),
            "zr": np.ascontiguousarray(zr[:, sl]),
            "zi": np.ascontiguousarray(zi[:, sl]),
            "wts": wts,
            "bias": bias,
            "xc": np.ascontiguousarray(xc[:, sl]),
            "ident": np.eye(P, dtype=f16),
        })
    return in_maps


def _gather_outputs(results):
    h_full = np.empty((B, P, H, W), np.float32)
    c_full = np.empty((B, P, H, W), np.float32)
    for c in range(N_CORES):
        sl = slice(c * B_CORE, (c + 1) * B_CORE)
        h_full[sl] = results[c]["h_out"].transpose(1, 0, 2, 3).astype(np.float32)
        c_full[sl] = results[c]["c_out"].transpose(1, 0, 2, 3).astype(np.float32)
    return h_full, c_full


def _run(inputs, trace=False, trace_kwargs=None):
    from concourse.bass_utils import run_bass_kernel_spmd

    if "nc" not in _CACHE:
        _CACHE["nc"] = _build_program()
    nc = _CACHE["nc"]
    in_maps = _prep_inputs(inputs)
    r = run_bass_kernel_spmd(nc, in_maps, list(range(N_CORES)),
                             trace=trace, trace_kwargs=trace_kwargs or {})
    return _gather_outputs(r.results), r


def kernel(**inputs):
    (h_full, c_full), _ = _run(inputs)
    return h_full, c_full
